# revision 5
# baseline (speedup 1.0000x reference)
"""GCN (3-layer graph conv) on 8 Trainium2 NeuronCores.

Strategy (graph/data parallel, dst-sharded):
- Nodes sharded across 8 cores (12500 each, padded to 12544 = 98 blocks of 128).
- Aggregation m[v] = sum_{(u,v) in E} h[u]: per 128-dst block, dma_gather
  fetches the src rows (bf16, 1KB each) in 128-edge tiles; segment-sum is a
  TensorE matmul with a DVE-built one-hot [128 edges x 128 dst] as the
  stationary operand, accumulating into one PSUM bank.
- Dense h@W: PE-transpose the scaled block then 4 K-block matmuls against the
  resident weight; epilogue fuses ReLU and the next layer's src-degree norm
  via ScalarE activation(scale=...).
- W_lin is folded into layer 0 (A(ns*X) @ (W_lin@W0) == A(ns*(X W_lin)) @ W0),
  so only 3 aggregation+dense rounds run on device.
- Inter-layer exchange: full h is rebuilt on every core by 7 chunked
  AllGathers per layer (14 blocks each) so they can overlap block compute.
- Gather tables use a [chunk][core][row] layout so each AllGather writes a
  contiguous slice; gather indices are window-relative int16 (4 windows of
  32768 rows), replicated across the 8 Q7 descriptor-generator stripes.
"""

import math
import numpy as np
import ml_dtypes

import concourse.bass as bass
import concourse.tile as tile
from concourse import bacc, mybir
from concourse.bass_utils import run_bass_kernel_spmd

BF16 = ml_dtypes.bfloat16


class Cfg:
    def __init__(self, N, E, D, C, win, blocks_per_chunk):
        self.N, self.E, self.D, self.C = N, E, D, C
        self.NC = 8
        assert N % self.NC == 0
        self.SH = N // self.NC                       # nodes per core
        self.BLK = math.ceil(self.SH / 128)          # 128-row blocks per core
        self.CHB = blocks_per_chunk                  # blocks per collective chunk
        assert self.BLK % self.CHB == 0
        self.NCH = self.BLK // self.CHB              # collective chunks
        self.PS = self.BLK * 128                     # padded shard rows
        self.CHR = self.CHB * 128                    # shard rows per chunk
        self.RT = self.NC * self.PS                  # full padded table rows
        self.WIN = win                               # gather window rows
        self.NW = math.ceil(self.RT / win)


CFG = Cfg(N=100000, E=3200000, D=512, C=40, win=32768, blocks_per_chunk=14)


def _rows_of(v, cfg):
    """Padded table row of node v under the [chunk][core][row] layout."""
    c = v // cfg.SH
    l = v % cfg.SH
    k = l // cfg.CHR
    r = l % cfg.CHR
    return (k * cfg.NC * cfg.CHR) + c * cfg.CHR + r


def _preprocess(cfg, features, src, dst, W_lin, b_lin, W0, b0, W1, b1, W2, b2):
    N, E, D, C = cfg.N, cfg.E, cfg.D, cfg.C
    deg_out = np.bincount(src, minlength=N).astype(np.float32)
    deg_in = np.bincount(dst, minlength=N).astype(np.float32)
    ns = np.maximum(deg_out, 1.0) ** -0.5
    nd = np.maximum(deg_in, 1.0) ** -0.5

    # gather table for layer 0: ns * X, padded layout, bf16
    rows = _rows_of(np.arange(N, dtype=np.int64), cfg)
    xt = np.zeros((cfg.RT, D), dtype=BF16)
    xt[rows] = (features * ns[:, None]).astype(BF16)

    # folded weights
    wc = (W_lin @ W0).astype(np.float32)

    def wtile(W, no):
        # [D, no] -> [128, D//128, no]
        return np.ascontiguousarray(
            W.reshape(D // 128, 128, no).transpose(1, 0, 2)
        ).astype(BF16)

    srow = _rows_of(src.astype(np.int64), cfg)
    swin = (srow // cfg.WIN).astype(np.int64)
    srel = (srow % cfg.WIN).astype(np.int64)

    # per-core edge grouping by (dst block, src window)
    per_core = []
    counts_all = np.zeros((cfg.NC, cfg.BLK * cfg.NW), dtype=np.int64)
    for c in range(cfg.NC):
        sel = (dst >= c * cfg.SH) & (dst < (c + 1) * cfg.SH)
        l = dst[sel].astype(np.int64) - c * cfg.SH
        b = l // 128
        code = (l % 128).astype(np.float32)
        w = swin[sel]
        rel = srel[sel]
        key = b * cfg.NW + w
        order = np.argsort(key, kind="stable")
        key_s = key[order]
        counts = np.bincount(key_s, minlength=cfg.BLK * cfg.NW)
        counts_all[c] = counts
        per_core.append((key_s, rel[order], code[order], counts))

    # global tile counts per (block, window)
    T_bw = np.ceil(counts_all.max(axis=0) / 128.0).astype(np.int64).reshape(cfg.BLK, cfg.NW)
    grp_pad = (T_bw.reshape(-1) * 128)                  # padded edges per group
    grp_off = np.concatenate([[0], np.cumsum(grp_pad)]) # element offsets
    Ltot = int(grp_off[-1])

    idx_arrs, code_arrs = [], []
    for c in range(cfg.NC):
        key_s, rel_s, code_s, counts = per_core[c]
        cnt_off = np.concatenate([[0], np.cumsum(counts)])
        # destination position for each (sorted) edge
        pos_in_grp = np.arange(len(key_s), dtype=np.int64) - cnt_off[key_s]
        destp = grp_off[key_s] + pos_in_grp
        idx_flat = np.zeros(Ltot, np.int16)
        code_flat = np.full(Ltot, 255.0, np.float32)
        idx_flat[destp] = rel_s.astype(np.int16)
        code_flat[destp] = code_s
        # wrap idx into 16 partitions, replicate across the 8 Q7 stripes
        idx16 = idx_flat.reshape(-1, 16).T                      # [16, L/16]
        idx_arrs.append(np.ascontiguousarray(np.tile(idx16, (8, 1))))
        code_arrs.append(np.ascontiguousarray(code_flat.reshape(-1, 128).T).astype(BF16))

    # per-core norm tiles [128, BLK]
    nd_t, ns_t = [], []
    for c in range(cfg.NC):
        pad = np.zeros(cfg.PS, np.float32)
        pad[: cfg.SH] = nd[c * cfg.SH : (c + 1) * cfg.SH]
        nd_t.append(np.ascontiguousarray(pad.reshape(cfg.BLK, 128).T))
        pad2 = np.zeros(cfg.PS, np.float32)
        pad2[: cfg.SH] = ns[c * cfg.SH : (c + 1) * cfg.SH]
        ns_t.append(np.ascontiguousarray(pad2.reshape(cfg.BLK, 128).T))

    iota = np.ascontiguousarray(
        np.broadcast_to(np.arange(128, dtype=np.float32)[None, :], (128, 128))
    ).astype(BF16)
    ident = np.eye(128, dtype=np.float32).astype(BF16)

    # bias terms (all-zero in this problem's spec; support nonzero anyway)
    bias = {}
    bias["b0row"] = np.asarray(b0, np.float32)
    bias["b1row"] = np.asarray(b1, np.float32)
    bias["b2row"] = np.asarray(b2, np.float32)
    bias["bw0row"] = (np.asarray(b_lin, np.float32) @ W0).astype(np.float32)
    if np.any(bias["bw0row"]):
        cvec = np.bincount(dst, weights=ns[src], minlength=N).astype(np.float32) * nd
        cn_t = []
        for c in range(cfg.NC):
            pad = np.zeros(cfg.PS, np.float32)
            pad[: cfg.SH] = cvec[c * cfg.SH : (c + 1) * cfg.SH]
            cn_t.append(np.ascontiguousarray(pad.reshape(cfg.BLK, 128).T))
        bias["cn_t"] = cn_t

    return dict(
        xt=xt,
        wc=wtile(wc, D), w1=wtile(np.asarray(W1, np.float32), D),
        w2=wtile(np.asarray(W2, np.float32), C),
        idx=idx_arrs, codes=code_arrs, nd=nd_t, ns=ns_t,
        iota=iota, ident=ident, T_bw=T_bw, Ltot=Ltot, bias=bias,
    )


def _build(cfg, T_bw, bias_en):
    """bias_en: dict of bools {b0, b1, b2, blin}."""
    D, C = cfg.D, cfg.C
    T_b = T_bw.sum(axis=1)                      # tiles per block
    tile_off = np.concatenate([[0], np.cumsum(T_b)])     # codes col offset per block
    grp_off = np.concatenate([[0], np.cumsum(T_bw.reshape(-1) * 128)])  # idx elem offsets
    Ltot = int(grp_off[-1])
    Ltiles = int(tile_off[-1])

    # per-chunk slices
    ch_tile0 = [int(tile_off[ch * cfg.CHB]) for ch in range(cfg.NCH + 1)]
    ch_idx0 = [int(grp_off[ch * cfg.CHB * cfg.NW]) for ch in range(cfg.NCH + 1)]
    max_ch_tiles = max(ch_tile0[i + 1] - ch_tile0[i] for i in range(cfg.NCH))
    max_ch_idx = max(ch_idx0[i + 1] - ch_idx0[i] for i in range(cfg.NCH))
    max_T_b = int(T_b.max())

    nc = bacc.Bacc("TRN2", num_devices=cfg.NC, num_swdge_queues=4)
    f32, bf16, i16 = mybir.dt.float32, mybir.dt.bfloat16, mybir.dt.int16

    xt = nc.declare_dram_parameter("xt", [cfg.RT, D], bf16, isOutput=False)
    idxs = nc.declare_dram_parameter("idxs", [128, Ltot // 16], i16, isOutput=False)
    codes = nc.declare_dram_parameter("codes", [128, Ltiles], bf16, isOutput=False)
    wc = nc.declare_dram_parameter("wc", [128, D // 128, D], bf16, isOutput=False)
    w1 = nc.declare_dram_parameter("w1", [128, D // 128, D], bf16, isOutput=False)
    w2 = nc.declare_dram_parameter("w2", [128, D // 128, C], bf16, isOutput=False)
    ndp = nc.declare_dram_parameter("nd", [128, cfg.BLK], f32, isOutput=False)
    nsp = nc.declare_dram_parameter("ns", [128, cfg.BLK], f32, isOutput=False)
    iota = nc.declare_dram_parameter("iota", [128, 128], bf16, isOutput=False)
    ident = nc.declare_dram_parameter("ident", [128, 128], bf16, isOutput=False)
    bias_p = {}
    if bias_en["b0"]:
        bias_p["b0row"] = nc.declare_dram_parameter("b0row", [128, D], f32, isOutput=False)
    if bias_en["b1"]:
        bias_p["b1row"] = nc.declare_dram_parameter("b1row", [128, D], f32, isOutput=False)
    if bias_en["b2"]:
        bias_p["b2row"] = nc.declare_dram_parameter("b2row", [128, C], f32, isOutput=False)
    if bias_en["blin"]:
        bias_p["bw0row"] = nc.declare_dram_parameter("bw0row", [128, D], f32, isOutput=False)
        bias_p["cn"] = nc.declare_dram_parameter("cn", [128, cfg.BLK], f32, isOutput=False)
    out = nc.declare_dram_parameter("out", [cfg.PS, C], f32, isOutput=True)

    h1s = nc.dram_tensor("h1s", [cfg.PS, D], bf16)
    h2s = nc.dram_tensor("h2s", [cfg.PS, D], bf16)
    h1f = nc.dram_tensor("h1f", [cfg.RT, D], bf16, addr_space="Shared")
    h2f = nc.dram_tensor("h2f", [cfg.RT, D], bf16, addr_space="Shared")

    cores = list(range(cfg.NC))

    with tile.TileContext(nc) as tc:
        with tc.tile_pool(name="const", bufs=1) as cp, \
             tc.tile_pool(name="chk", bufs=2) as kp, \
             tc.tile_pool(name="stag", bufs=2) as sp, \
             tc.tile_pool(name="work", bufs=3) as wp, \
             tc.tile_pool(name="psA", bufs=2, space="PSUM") as psA, \
             tc.tile_pool(name="psT", bufs=2, space="PSUM") as psT, \
             tc.tile_pool(name="psD", bufs=2, space="PSUM") as psD:

            iota_sb = cp.tile([128, 128], bf16)
            nc.sync.dma_start(out=iota_sb[:], in_=iota[:])
            ident_sb = cp.tile([128, 128], bf16)
            nc.sync.dma_start(out=ident_sb[:], in_=ident[:])
            nd_sb = cp.tile([128, cfg.BLK], f32)
            nc.sync.dma_start(out=nd_sb[:], in_=ndp[:])
            ns_sb = cp.tile([128, cfg.BLK], f32)
            nc.sync.dma_start(out=ns_sb[:], in_=nsp[:])
            w_sb = {}
            for name, par, no in (("wc", wc, D), ("w1", w1, D), ("w2", w2, C)):
                t = cp.tile([128, D // 128, no], bf16, tag=f"w_{name}")
                nc.sync.dma_start(out=t[:], in_=par[:])
                w_sb[name] = t
            bias_sb = {}
            for nm in ("b0row", "b1row", "b2row", "bw0row"):
                if nm in bias_p:
                    no = C if nm == "b2row" else D
                    t = cp.tile([128, no], f32, tag=f"bias_{nm}")
                    nc.sync.dma_start(out=t[:], in_=bias_p[nm][:])
                    bias_sb[nm] = t
            if "cn" in bias_p:
                t = cp.tile([128, cfg.BLK], f32)
                nc.sync.dma_start(out=t[:], in_=bias_p["cn"][:])
                bias_sb["cn"] = t

            self_qn = [0]
            layers = (
                ("wc", xt, h1s, h1f, True, "b0row"),
                ("w1", h1f, h2s, h2f, True, "b1row"),
                ("w2", h2f, None, None, False, "b2row"),
            )
            for li, (wname, srct, hshard, hfull, relu_ns, brow) in enumerate(layers):
                NO = D if li < 2 else C
                wt = w_sb[wname]
                for ch in range(cfg.NCH):
                    nt = ch_tile0[ch + 1] - ch_tile0[ch]
                    ni = ch_idx0[ch + 1] - ch_idx0[ch]
                    code_ch = kp.tile([128, max_ch_tiles], bf16, tag="code_ch")
                    nc.sync.dma_start(
                        out=code_ch[:, :nt], in_=codes[:, ch_tile0[ch] : ch_tile0[ch + 1]]
                    )
                    idx_ch = kp.tile([128, max_ch_idx // 16], i16, tag="idx_ch")
                    nc.sync.dma_start(
                        out=idx_ch[:, : ni // 16],
                        in_=idxs[:, ch_idx0[ch] // 16 : ch_idx0[ch + 1] // 16],
                    )
                    for bl in range(cfg.CHB):
                        b = ch * cfg.CHB + bl
                        tb = int(T_b[b])
                        stag = sp.tile([128, max_T_b, D], bf16, tag="stag")
                        o = 0
                        for w in range(cfg.NW):
                            tw = int(T_bw[b, w])
                            if tw == 0:
                                continue
                            i0 = (int(grp_off[b * cfg.NW + w]) - ch_idx0[ch]) // 16
                            wlo = w * cfg.WIN
                            whi = min(wlo + cfg.WIN, cfg.RT)
                            # dma_gather caps at 1024 idxs (128 descs x 8 Q7
                            # cores); split and round-robin the 4 SWDGE queues
                            for t0 in range(0, tw, 8):
                                tn = min(8, tw - t0)
                                nc.gpsimd.dma_gather(
                                    out_ap=stag[:, o + t0 : o + t0 + tn, :],
                                    in_ap=srct[wlo:whi, :],
                                    idxs_ap=idx_ch[:, i0 + t0 * 8 : i0 + (t0 + tn) * 8],
                                    num_idxs=tn * 128,
                                    num_idxs_reg=tn * 128,
                                    elem_size=D,
                                    queue_num=self_qn[0] % 4,
                                )
                                self_qn[0] += 1
                            o += tw
                        acc = psA.tile([128, D], f32, space="PSUM", tag="acc")
                        cb = int(tile_off[b]) - ch_tile0[ch]
                        for ti in range(tb):
                            oh = wp.tile([128, 128], bf16, tag="oh")
                            nc.vector.tensor_tensor(
                                out=oh[:],
                                in0=code_ch[:, cb + ti : cb + ti + 1].to_broadcast([128, 128]),
                                in1=iota_sb[:],
                                op=mybir.AluOpType.is_equal,
                            )
                            nc.tensor.matmul(
                                out=acc[:], lhsT=oh[:], rhs=stag[:, ti, :],
                                start=(ti == 0), stop=(ti == tb - 1),
                            )
                        # scale by dst-degree norm, cast bf16
                        mn = wp.tile([128, D], bf16, tag="mn")
                        nc.vector.tensor_scalar_mul(mn[:], acc[:], nd_sb[:, b : b + 1])
                        # transpose mn -> [f_in, nodes] chunks
                        pT = psT.tile([128, D], bf16, space="PSUM", tag="pT")
                        for j in range(D // 128):
                            nc.tensor.transpose(
                                out=pT[:, j * 128 : (j + 1) * 128],
                                in_=mn[:, j * 128 : (j + 1) * 128],
                                identity=ident_sb[:],
                            )
                        lhsT = wp.tile([128, D], bf16, tag="lhsT")
                        nc.vector.tensor_copy(out=lhsT[:], in_=pT[:])
                        pd = psD.tile([128, NO], f32, space="PSUM", tag="pd")
                        for j in range(D // 128):
                            nc.tensor.matmul(
                                out=pd[:], lhsT=lhsT[:, j * 128 : (j + 1) * 128],
                                rhs=wt[:, j, :], start=(j == 0), stop=(j == D // 128 - 1),
                            )
                        # epilogue
                        pre = pd
                        if li == 0 and "bw0row" in bias_sb:
                            tmp = wp.tile([128, D], f32, tag="btmp")
                            nc.vector.tensor_scalar_mul(
                                tmp[:], bias_sb["bw0row"][:], bias_sb["cn"][:, b : b + 1]
                            )
                            nc.vector.tensor_add(pd[:], pd[:], tmp[:])
                        if brow in bias_sb:
                            nc.vector.tensor_add(pd[:], pd[:], bias_sb[brow][:])
                        if relu_ns:
                            ht = wp.tile([128, D], bf16, tag="ht")
                            nc.scalar.activation(
                                ht[:], pre[:],
                                mybir.ActivationFunctionType.Relu,
                                scale=ns_sb[:, b : b + 1],
                            )
                            nc.sync.dma_start(
                                out=hshard[b * 128 : (b + 1) * 128, :], in_=ht[:]
                            )
                        else:
                            ot = wp.tile([128, C], f32, tag="ot")
                            nc.vector.tensor_copy(out=ot[:], in_=pd[:])
                            nc.sync.dma_start(
                                out=out[b * 128 : (b + 1) * 128, :], in_=ot[:]
                            )
                    if li < 2:
                        nc.gpsimd.collective_compute(
                            "AllGather",
                            mybir.AluOpType.bypass,
                            ins=[hshard[ch * cfg.CHR : (ch + 1) * cfg.CHR, :]],
                            outs=[
                                hfull[
                                    ch * cfg.NC * cfg.CHR : (ch + 1) * cfg.NC * cfg.CHR, :
                                ]
                            ],
                            replica_groups=[cores],
                        )
    nc.compile()
    return nc


_CACHE = {}


def _run(cfg, inputs, trace=False):
    pre = _preprocess(cfg, **inputs)
    bias_en = {
        "b0": bool(np.any(pre["bias"]["b0row"])),
        "b1": bool(np.any(pre["bias"]["b1row"])),
        "b2": bool(np.any(pre["bias"]["b2row"])),
        "blin": bool(np.any(pre["bias"]["bw0row"])),
    }
    key = (id(cfg), tuple(pre["T_bw"].reshape(-1)), tuple(sorted(bias_en.items())))
    if key not in _CACHE:
        _CACHE[key] = _build(cfg, pre["T_bw"], bias_en)
    nc = _CACHE[key]

    in_maps = []
    for c in range(cfg.NC):
        m = dict(
            xt=pre["xt"],
            idxs=pre["idx"][c], codes=pre["codes"][c],
            wc=pre["wc"], w1=pre["w1"], w2=pre["w2"],
            nd=pre["nd"][c], ns=pre["ns"][c],
            iota=pre["iota"], ident=pre["ident"],
        )
        rep = lambda v: np.ascontiguousarray(np.broadcast_to(v[None, :], (128, v.shape[0])))
        if bias_en["b0"]:
            m["b0row"] = rep(pre["bias"]["b0row"])
        if bias_en["b1"]:
            m["b1row"] = rep(pre["bias"]["b1row"])
        if bias_en["b2"]:
            m["b2row"] = rep(pre["bias"]["b2row"])
        if bias_en["blin"]:
            m["bw0row"] = rep(pre["bias"]["bw0row"])
            m["cn"] = pre["bias"]["cn_t"][c]
        in_maps.append(m)

    r = run_bass_kernel_spmd(nc, in_maps, list(range(cfg.NC)), trace=trace)
    outs = [np.asarray(r.results[c]["out"])[: cfg.SH] for c in range(cfg.NC)]
    full = np.concatenate(outs, axis=0)[: cfg.N]
    return full, r


def kernel(**inputs):
    inputs = {k: np.asarray(v) for k, v in inputs.items()}
    out, _ = _run(CFG, inputs)
    return out


# revision 7
# speedup vs baseline: 1.0004x; 1.0004x over previous
"""GCN (3-layer graph conv) on 8 Trainium2 NeuronCores.

Strategy (graph/data parallel, dst-sharded):
- Nodes sharded across 8 cores (12500 each, padded to 12544 = 98 blocks of 128).
- Aggregation m[v] = sum_{(u,v) in E} h[u]: per 128-dst block, dma_gather
  fetches the src rows (bf16, 1KB each) in 128-edge tiles; segment-sum is a
  TensorE matmul with a DVE-built one-hot [128 edges x 128 dst] as the
  stationary operand, accumulating into one PSUM bank.
- Dense h@W: PE-transpose the scaled block then 4 K-block matmuls against the
  resident weight; epilogue fuses ReLU and the next layer's src-degree norm
  via ScalarE activation(scale=...).
- W_lin is folded into layer 0 (A(ns*X) @ (W_lin@W0) == A(ns*(X W_lin)) @ W0),
  so only 3 aggregation+dense rounds run on device.
- Inter-layer exchange: full h is rebuilt on every core by 7 chunked
  AllGathers per layer (14 blocks each) so they can overlap block compute.
- Gather tables use a [chunk][core][row] layout so each AllGather writes a
  contiguous slice; gather indices are window-relative int16 (4 windows of
  32768 rows), replicated across the 8 Q7 descriptor-generator stripes.
"""

import math
import numpy as np
import ml_dtypes

import concourse.bass as bass
import concourse.tile as tile
from concourse import bacc, mybir
from concourse.bass_utils import run_bass_kernel_spmd

BF16 = ml_dtypes.bfloat16


class Cfg:
    def __init__(self, N, E, D, C, win, blocks_per_chunk):
        self.N, self.E, self.D, self.C = N, E, D, C
        self.NC = 8
        assert N % self.NC == 0
        self.SH = N // self.NC                       # nodes per core
        self.BLK = math.ceil(self.SH / 128)          # 128-row blocks per core
        self.CHB = blocks_per_chunk                  # blocks per collective chunk
        assert self.BLK % self.CHB == 0
        self.NCH = self.BLK // self.CHB              # collective chunks
        self.PS = self.BLK * 128                     # padded shard rows
        self.CHR = self.CHB * 128                    # shard rows per chunk
        self.RT = self.NC * self.PS                  # full padded table rows
        self.WIN = win                               # gather window rows
        self.NW = math.ceil(self.RT / win)


CFG = Cfg(N=100000, E=3200000, D=512, C=40, win=32768, blocks_per_chunk=14)


def _rows_of(v, cfg):
    """Padded table row of node v under the [chunk][core][row] layout."""
    c = v // cfg.SH
    l = v % cfg.SH
    k = l // cfg.CHR
    r = l % cfg.CHR
    return (k * cfg.NC * cfg.CHR) + c * cfg.CHR + r


def _preprocess(cfg, features, src, dst, W_lin, b_lin, W0, b0, W1, b1, W2, b2):
    N, E, D, C = cfg.N, cfg.E, cfg.D, cfg.C
    deg_out = np.bincount(src, minlength=N).astype(np.float32)
    deg_in = np.bincount(dst, minlength=N).astype(np.float32)
    ns = np.maximum(deg_out, 1.0) ** -0.5
    nd = np.maximum(deg_in, 1.0) ** -0.5

    # gather table for layer 0: ns * X, padded layout, bf16
    rows = _rows_of(np.arange(N, dtype=np.int64), cfg)
    xt = np.zeros((cfg.RT, D), dtype=BF16)
    xt[rows] = (features * ns[:, None]).astype(BF16)

    # folded weights
    wc = (W_lin @ W0).astype(np.float32)

    def wtile(W, no):
        # [D, no] -> [128, D//128, no]
        return np.ascontiguousarray(
            W.reshape(D // 128, 128, no).transpose(1, 0, 2)
        ).astype(BF16)

    srow = _rows_of(src.astype(np.int64), cfg)
    swin = (srow // cfg.WIN).astype(np.int64)
    srel = (srow % cfg.WIN).astype(np.int64)

    # per-core edge grouping by (dst block, src window)
    per_core = []
    counts_all = np.zeros((cfg.NC, cfg.BLK * cfg.NW), dtype=np.int64)
    for c in range(cfg.NC):
        sel = (dst >= c * cfg.SH) & (dst < (c + 1) * cfg.SH)
        l = dst[sel].astype(np.int64) - c * cfg.SH
        b = l // 128
        code = (l % 128).astype(np.float32)
        w = swin[sel]
        rel = srel[sel]
        key = b * cfg.NW + w
        order = np.argsort(key, kind="stable")
        key_s = key[order]
        counts = np.bincount(key_s, minlength=cfg.BLK * cfg.NW)
        counts_all[c] = counts
        per_core.append((key_s, rel[order], code[order], counts))

    # global tile counts per (block, window)
    T_bw = np.ceil(counts_all.max(axis=0) / 128.0).astype(np.int64).reshape(cfg.BLK, cfg.NW)
    grp_pad = (T_bw.reshape(-1) * 128)                  # padded edges per group
    grp_off = np.concatenate([[0], np.cumsum(grp_pad)]) # element offsets
    Ltot = int(grp_off[-1])

    idx_arrs, code_arrs = [], []
    for c in range(cfg.NC):
        key_s, rel_s, code_s, counts = per_core[c]
        cnt_off = np.concatenate([[0], np.cumsum(counts)])
        # destination position for each (sorted) edge
        pos_in_grp = np.arange(len(key_s), dtype=np.int64) - cnt_off[key_s]
        destp = grp_off[key_s] + pos_in_grp
        idx_flat = np.zeros(Ltot, np.int16)
        code_flat = np.full(Ltot, 255.0, np.float32)
        idx_flat[destp] = rel_s.astype(np.int16)
        code_flat[destp] = code_s
        # wrap idx into 16 partitions, replicate across the 8 Q7 stripes
        idx16 = idx_flat.reshape(-1, 16).T                      # [16, L/16]
        idx_arrs.append(np.ascontiguousarray(np.tile(idx16, (8, 1))))
        code_arrs.append(np.ascontiguousarray(code_flat.reshape(-1, 128).T).astype(BF16))

    # per-core norm tiles [128, BLK]
    nd_t, ns_t = [], []
    for c in range(cfg.NC):
        pad = np.zeros(cfg.PS, np.float32)
        pad[: cfg.SH] = nd[c * cfg.SH : (c + 1) * cfg.SH]
        nd_t.append(np.ascontiguousarray(pad.reshape(cfg.BLK, 128).T))
        pad2 = np.zeros(cfg.PS, np.float32)
        pad2[: cfg.SH] = ns[c * cfg.SH : (c + 1) * cfg.SH]
        ns_t.append(np.ascontiguousarray(pad2.reshape(cfg.BLK, 128).T))

    iota = np.ascontiguousarray(
        np.broadcast_to(
            np.tile(np.arange(128, dtype=np.float32), 4)[None, :], (128, 512)
        )
    ).astype(BF16)
    ident = np.eye(128, dtype=np.float32).astype(BF16)

    # bias terms (all-zero in this problem's spec; support nonzero anyway)
    bias = {}
    bias["b0row"] = np.asarray(b0, np.float32)
    bias["b1row"] = np.asarray(b1, np.float32)
    bias["b2row"] = np.asarray(b2, np.float32)
    bias["bw0row"] = (np.asarray(b_lin, np.float32) @ W0).astype(np.float32)
    if np.any(bias["bw0row"]):
        cvec = np.bincount(dst, weights=ns[src], minlength=N).astype(np.float32) * nd
        cn_t = []
        for c in range(cfg.NC):
            pad = np.zeros(cfg.PS, np.float32)
            pad[: cfg.SH] = cvec[c * cfg.SH : (c + 1) * cfg.SH]
            cn_t.append(np.ascontiguousarray(pad.reshape(cfg.BLK, 128).T))
        bias["cn_t"] = cn_t

    return dict(
        xt=xt,
        wc=wtile(wc, D), w1=wtile(np.asarray(W1, np.float32), D),
        w2=wtile(np.asarray(W2, np.float32), C),
        idx=idx_arrs, codes=code_arrs, nd=nd_t, ns=ns_t,
        iota=iota, ident=ident, T_bw=T_bw, Ltot=Ltot, bias=bias,
    )


def _build(cfg, T_bw, bias_en):
    """bias_en: dict of bools {b0, b1, b2, blin}."""
    D, C = cfg.D, cfg.C
    T_b = T_bw.sum(axis=1)                      # tiles per block
    tile_off = np.concatenate([[0], np.cumsum(T_b)])     # codes col offset per block
    grp_off = np.concatenate([[0], np.cumsum(T_bw.reshape(-1) * 128)])  # idx elem offsets
    Ltot = int(grp_off[-1])
    Ltiles = int(tile_off[-1])

    # per-chunk slices
    ch_tile0 = [int(tile_off[ch * cfg.CHB]) for ch in range(cfg.NCH + 1)]
    ch_idx0 = [int(grp_off[ch * cfg.CHB * cfg.NW]) for ch in range(cfg.NCH + 1)]
    max_ch_tiles = max(ch_tile0[i + 1] - ch_tile0[i] for i in range(cfg.NCH))
    max_ch_idx = max(ch_idx0[i + 1] - ch_idx0[i] for i in range(cfg.NCH))
    max_T_b = int(T_b.max())

    nc = bacc.Bacc("TRN2", num_devices=cfg.NC, num_swdge_queues=4)
    f32, bf16, i16 = mybir.dt.float32, mybir.dt.bfloat16, mybir.dt.int16

    xt = nc.declare_dram_parameter("xt", [cfg.RT, D], bf16, isOutput=False)
    idxs = nc.declare_dram_parameter("idxs", [128, Ltot // 16], i16, isOutput=False)
    codes = nc.declare_dram_parameter("codes", [128, Ltiles], bf16, isOutput=False)
    wc = nc.declare_dram_parameter("wc", [128, D // 128, D], bf16, isOutput=False)
    w1 = nc.declare_dram_parameter("w1", [128, D // 128, D], bf16, isOutput=False)
    w2 = nc.declare_dram_parameter("w2", [128, D // 128, C], bf16, isOutput=False)
    ndp = nc.declare_dram_parameter("nd", [128, cfg.BLK], f32, isOutput=False)
    nsp = nc.declare_dram_parameter("ns", [128, cfg.BLK], f32, isOutput=False)
    iota = nc.declare_dram_parameter("iota", [128, 4, 128], bf16, isOutput=False)
    ident = nc.declare_dram_parameter("ident", [128, 128], bf16, isOutput=False)
    bias_p = {}
    if bias_en["b0"]:
        bias_p["b0row"] = nc.declare_dram_parameter("b0row", [128, D], f32, isOutput=False)
    if bias_en["b1"]:
        bias_p["b1row"] = nc.declare_dram_parameter("b1row", [128, D], f32, isOutput=False)
    if bias_en["b2"]:
        bias_p["b2row"] = nc.declare_dram_parameter("b2row", [128, C], f32, isOutput=False)
    if bias_en["blin"]:
        bias_p["bw0row"] = nc.declare_dram_parameter("bw0row", [128, D], f32, isOutput=False)
        bias_p["cn"] = nc.declare_dram_parameter("cn", [128, cfg.BLK], f32, isOutput=False)
    out = nc.declare_dram_parameter("out", [cfg.PS, C], f32, isOutput=True)

    h1s = [nc.dram_tensor(f"h1s_{i}", [cfg.CHR, D], bf16) for i in range(cfg.NCH)]
    h2s = [nc.dram_tensor(f"h2s_{i}", [cfg.CHR, D], bf16) for i in range(cfg.NCH)]
    h1f = nc.dram_tensor("h1f", [cfg.RT, D], bf16, addr_space="Shared")
    h2f = nc.dram_tensor("h2f", [cfg.RT, D], bf16, addr_space="Shared")

    cores = list(range(cfg.NC))

    with tile.TileContext(nc) as tc:
        with tc.tile_pool(name="const", bufs=1) as cp, \
             tc.tile_pool(name="chk", bufs=2) as kp, \
             tc.tile_pool(name="stag", bufs=2) as sp, \
             tc.tile_pool(name="work", bufs=3) as wp, \
             tc.tile_pool(name="psA", bufs=2, space="PSUM") as psA, \
             tc.tile_pool(name="psT", bufs=2, space="PSUM") as psT, \
             tc.tile_pool(name="psD", bufs=2, space="PSUM") as psD:

            iota_sb = cp.tile([128, 4, 128], bf16)
            nc.sync.dma_start(out=iota_sb[:], in_=iota[:])
            ident_sb = cp.tile([128, 128], bf16)
            nc.sync.dma_start(out=ident_sb[:], in_=ident[:])
            nd_sb = cp.tile([128, cfg.BLK], f32)
            nc.sync.dma_start(out=nd_sb[:], in_=ndp[:])
            ns_sb = cp.tile([128, cfg.BLK], f32)
            nc.sync.dma_start(out=ns_sb[:], in_=nsp[:])
            w_sb = {}
            for name, par, no in (("wc", wc, D), ("w1", w1, D), ("w2", w2, C)):
                t = cp.tile([128, D // 128, no], bf16, tag=f"w_{name}")
                nc.sync.dma_start(out=t[:], in_=par[:])
                w_sb[name] = t
            bias_sb = {}
            for nm in ("b0row", "b1row", "b2row", "bw0row"):
                if nm in bias_p:
                    no = C if nm == "b2row" else D
                    t = cp.tile([128, no], f32, tag=f"bias_{nm}")
                    nc.sync.dma_start(out=t[:], in_=bias_p[nm][:])
                    bias_sb[nm] = t
            if "cn" in bias_p:
                t = cp.tile([128, cfg.BLK], f32)
                nc.sync.dma_start(out=t[:], in_=bias_p["cn"][:])
                bias_sb["cn"] = t

            self_qn = [0]
            layers = (
                ("wc", xt, h1s, h1f, True, "b0row"),
                ("w1", h1f, h2s, h2f, True, "b1row"),
                ("w2", h2f, None, None, False, "b2row"),
            )
            for li, (wname, srct, hshard, hfull, relu_ns, brow) in enumerate(layers):
                NO = D if li < 2 else C
                wt = w_sb[wname]
                for ch in range(cfg.NCH):
                    nt = ch_tile0[ch + 1] - ch_tile0[ch]
                    ni = ch_idx0[ch + 1] - ch_idx0[ch]
                    code_ch = kp.tile([128, max_ch_tiles], bf16, tag="code_ch")
                    nc.sync.dma_start(
                        out=code_ch[:, :nt], in_=codes[:, ch_tile0[ch] : ch_tile0[ch + 1]]
                    )
                    idx_ch = kp.tile([128, max_ch_idx // 16], i16, tag="idx_ch")
                    nc.sync.dma_start(
                        out=idx_ch[:, : ni // 16],
                        in_=idxs[:, ch_idx0[ch] // 16 : ch_idx0[ch + 1] // 16],
                    )
                    for bl in range(cfg.CHB):
                        b = ch * cfg.CHB + bl
                        tb = int(T_b[b])
                        stag = sp.tile([128, max_T_b, D], bf16, tag="stag")
                        o = 0
                        for w in range(cfg.NW):
                            tw = int(T_bw[b, w])
                            if tw == 0:
                                continue
                            i0 = (int(grp_off[b * cfg.NW + w]) - ch_idx0[ch]) // 16
                            wlo = w * cfg.WIN
                            whi = min(wlo + cfg.WIN, cfg.RT)
                            # dma_gather caps at 1024 idxs (128 descs x 8 Q7
                            # cores); split and round-robin the 4 SWDGE queues
                            for t0 in range(0, tw, 8):
                                tn = min(8, tw - t0)
                                nc.gpsimd.dma_gather(
                                    out_ap=stag[:, o + t0 : o + t0 + tn, :],
                                    in_ap=srct[wlo:whi, :],
                                    idxs_ap=idx_ch[:, i0 + t0 * 8 : i0 + (t0 + tn) * 8],
                                    num_idxs=tn * 128,
                                    num_idxs_reg=tn * 128,
                                    elem_size=D,
                                    queue_num=self_qn[0] % 4,
                                )
                                self_qn[0] += 1
                            o += tw
                        acc = psA.tile([128, D], f32, space="PSUM", tag="acc")
                        cb = int(tile_off[b]) - ch_tile0[ch]
                        for g in range(0, tb, 4):
                            gn = min(4, tb - g)
                            oh = wp.tile([128, 4, 128], bf16, tag="oh")
                            nc.vector.tensor_tensor(
                                out=oh[:, :gn, :],
                                in0=code_ch[:, cb + g : cb + g + gn].to_broadcast(
                                    [128, gn, 128]
                                ),
                                in1=iota_sb[:, :gn, :],
                                op=mybir.AluOpType.is_equal,
                            )
                            for k in range(gn):
                                ti = g + k
                                nc.tensor.matmul(
                                    out=acc[:], lhsT=oh[:, k, :], rhs=stag[:, ti, :],
                                    start=(ti == 0), stop=(ti == tb - 1),
                                )
                        # scale by dst-degree norm, cast bf16
                        mn = wp.tile([128, D], bf16, tag="mn")
                        nc.vector.tensor_scalar_mul(mn[:], acc[:], nd_sb[:, b : b + 1])
                        # transpose mn -> [f_in, nodes] chunks
                        pT = psT.tile([128, D], bf16, space="PSUM", tag="pT")
                        for j in range(D // 128):
                            nc.tensor.transpose(
                                out=pT[:, j * 128 : (j + 1) * 128],
                                in_=mn[:, j * 128 : (j + 1) * 128],
                                identity=ident_sb[:],
                            )
                        lhsT = wp.tile([128, D], bf16, tag="lhsT")
                        nc.vector.tensor_copy(out=lhsT[:], in_=pT[:])
                        pd = psD.tile([128, NO], f32, space="PSUM", tag="pd")
                        for j in range(D // 128):
                            nc.tensor.matmul(
                                out=pd[:], lhsT=lhsT[:, j * 128 : (j + 1) * 128],
                                rhs=wt[:, j, :], start=(j == 0), stop=(j == D // 128 - 1),
                            )
                        # epilogue
                        pre = pd
                        if li == 0 and "bw0row" in bias_sb:
                            tmp = wp.tile([128, D], f32, tag="btmp")
                            nc.vector.tensor_scalar_mul(
                                tmp[:], bias_sb["bw0row"][:], bias_sb["cn"][:, b : b + 1]
                            )
                            nc.vector.tensor_add(pd[:], pd[:], tmp[:])
                        if brow in bias_sb:
                            nc.vector.tensor_add(pd[:], pd[:], bias_sb[brow][:])
                        if relu_ns:
                            ht = wp.tile([128, D], bf16, tag="ht")
                            nc.scalar.activation(
                                ht[:], pre[:],
                                mybir.ActivationFunctionType.Relu,
                                scale=ns_sb[:, b : b + 1],
                            )
                            nc.sync.dma_start(
                                out=hshard[ch][bl * 128 : (bl + 1) * 128, :], in_=ht[:]
                            )
                        else:
                            ot = wp.tile([128, C], f32, tag="ot")
                            nc.vector.tensor_copy(out=ot[:], in_=pd[:])
                            nc.sync.dma_start(
                                out=out[b * 128 : (b + 1) * 128, :], in_=ot[:]
                            )
                    if li < 2:
                        nc.gpsimd.collective_compute(
                            "AllGather",
                            mybir.AluOpType.bypass,
                            ins=[hshard[ch][:]],
                            outs=[
                                hfull[
                                    ch * cfg.NC * cfg.CHR : (ch + 1) * cfg.NC * cfg.CHR, :
                                ]
                            ],
                            replica_groups=[cores],
                        )
    nc.compile()
    return nc


_CACHE = {}


def _run(cfg, inputs, trace=False):
    pre = _preprocess(cfg, **inputs)
    bias_en = {
        "b0": bool(np.any(pre["bias"]["b0row"])),
        "b1": bool(np.any(pre["bias"]["b1row"])),
        "b2": bool(np.any(pre["bias"]["b2row"])),
        "blin": bool(np.any(pre["bias"]["bw0row"])),
    }
    key = (id(cfg), tuple(pre["T_bw"].reshape(-1)), tuple(sorted(bias_en.items())))
    if key not in _CACHE:
        _CACHE[key] = _build(cfg, pre["T_bw"], bias_en)
    nc = _CACHE[key]

    in_maps = []
    for c in range(cfg.NC):
        m = dict(
            xt=pre["xt"],
            idxs=pre["idx"][c], codes=pre["codes"][c],
            wc=pre["wc"], w1=pre["w1"], w2=pre["w2"],
            nd=pre["nd"][c], ns=pre["ns"][c],
            iota=pre["iota"], ident=pre["ident"],
        )
        rep = lambda v: np.ascontiguousarray(np.broadcast_to(v[None, :], (128, v.shape[0])))
        if bias_en["b0"]:
            m["b0row"] = rep(pre["bias"]["b0row"])
        if bias_en["b1"]:
            m["b1row"] = rep(pre["bias"]["b1row"])
        if bias_en["b2"]:
            m["b2row"] = rep(pre["bias"]["b2row"])
        if bias_en["blin"]:
            m["bw0row"] = rep(pre["bias"]["bw0row"])
            m["cn"] = pre["bias"]["cn_t"][c]
        in_maps.append(m)

    r = run_bass_kernel_spmd(nc, in_maps, list(range(cfg.NC)), trace=trace)
    outs = [np.asarray(r.results[c]["out"])[: cfg.SH] for c in range(cfg.NC)]
    full = np.concatenate(outs, axis=0)[: cfg.N]
    return full, r


def kernel(**inputs):
    inputs = {k: np.asarray(v) for k, v in inputs.items()}
    out, _ = _run(CFG, inputs)
    return out


# revision 8
# speedup vs baseline: 1.3297x; 1.3292x over previous
"""GCN (3-layer graph conv) on 8 Trainium2 NeuronCores.

Strategy (graph/data parallel, dst-sharded):
- Nodes sharded across 8 cores (12500 each, padded to 12544 = 98 blocks of 128).
- Aggregation m[v] = sum_{(u,v) in E} h[u]: per 128-dst block, dma_gather
  fetches the src rows (bf16, 1KB each) in 128-edge tiles; segment-sum is a
  TensorE matmul with a DVE-built one-hot [128 edges x 128 dst] as the
  stationary operand, accumulating into one PSUM bank.
- Dense h@W: PE-transpose the scaled block then 4 K-block matmuls against the
  resident weight; epilogue fuses ReLU and the next layer's src-degree norm
  via ScalarE activation(scale=...).
- W_lin is folded into layer 0 (A(ns*X) @ (W_lin@W0) == A(ns*(X W_lin)) @ W0),
  so only 3 aggregation+dense rounds run on device.
- Inter-layer exchange: full h is rebuilt on every core by 7 chunked
  AllGathers per layer (14 blocks each) so they can overlap block compute.
- Gather tables use a [chunk][core][row] layout so each AllGather writes a
  contiguous slice; gather indices are window-relative int16 (4 windows of
  32768 rows), replicated across the 8 Q7 descriptor-generator stripes.
"""

import math
import numpy as np
import ml_dtypes

import concourse.bass as bass
import concourse.tile as tile
from concourse import bacc, mybir
from concourse.bass_utils import run_bass_kernel_spmd

BF16 = ml_dtypes.bfloat16


class Cfg:
    def __init__(self, N, E, D, C, win, blocks_per_chunk):
        self.N, self.E, self.D, self.C = N, E, D, C
        self.NC = 8
        assert N % self.NC == 0
        self.SH = N // self.NC                       # nodes per core
        self.BLK = math.ceil(self.SH / 128)          # 128-row blocks per core
        self.CHB = blocks_per_chunk                  # blocks per collective chunk
        assert self.BLK % self.CHB == 0
        self.NCH = self.BLK // self.CHB              # collective chunks
        self.PS = self.BLK * 128                     # padded shard rows
        self.CHR = self.CHB * 128                    # shard rows per chunk
        self.RT = self.NC * self.PS                  # full padded table rows
        self.WIN = win                               # gather window rows
        self.NW = math.ceil(self.RT / win)


CFG = Cfg(N=100000, E=3200000, D=512, C=40, win=32768, blocks_per_chunk=14)


def _rows_of(v, cfg):
    """Padded table row of node v under the [chunk][core][row] layout."""
    c = v // cfg.SH
    l = v % cfg.SH
    k = l // cfg.CHR
    r = l % cfg.CHR
    return (k * cfg.NC * cfg.CHR) + c * cfg.CHR + r


def _preprocess(cfg, features, src, dst, W_lin, b_lin, W0, b0, W1, b1, W2, b2):
    N, E, D, C = cfg.N, cfg.E, cfg.D, cfg.C
    deg_out = np.bincount(src, minlength=N).astype(np.float32)
    deg_in = np.bincount(dst, minlength=N).astype(np.float32)
    ns = np.maximum(deg_out, 1.0) ** -0.5
    nd = np.maximum(deg_in, 1.0) ** -0.5

    # gather table for layer 0: ns * X, padded layout, bf16
    rows = _rows_of(np.arange(N, dtype=np.int64), cfg)
    xt = np.zeros((cfg.RT, D), dtype=BF16)
    xt[rows] = (features * ns[:, None]).astype(BF16)

    # folded weights
    wc = (W_lin @ W0).astype(np.float32)

    def wtile(W, no):
        # [D, no] -> [128, D//128, no]
        return np.ascontiguousarray(
            W.reshape(D // 128, 128, no).transpose(1, 0, 2)
        ).astype(BF16)

    srow = _rows_of(src.astype(np.int64), cfg)
    swin = (srow // cfg.WIN).astype(np.int64)
    srel = (srow % cfg.WIN).astype(np.int64)

    # per-core edge grouping by (dst block, src window)
    per_core = []
    counts_all = np.zeros((cfg.NC, cfg.BLK * cfg.NW), dtype=np.int64)
    for c in range(cfg.NC):
        sel = (dst >= c * cfg.SH) & (dst < (c + 1) * cfg.SH)
        l = dst[sel].astype(np.int64) - c * cfg.SH
        b = l // 128
        code = (l % 128).astype(np.float32)
        w = swin[sel]
        rel = srel[sel]
        key = b * cfg.NW + w
        order = np.argsort(key, kind="stable")
        key_s = key[order]
        counts = np.bincount(key_s, minlength=cfg.BLK * cfg.NW)
        counts_all[c] = counts
        per_core.append((key_s, rel[order], code[order], counts))

    # global tile counts per (block, window)
    T_bw = np.ceil(counts_all.max(axis=0) / 128.0).astype(np.int64).reshape(cfg.BLK, cfg.NW)
    grp_pad = (T_bw.reshape(-1) * 128)                  # padded edges per group
    grp_off = np.concatenate([[0], np.cumsum(grp_pad)]) # element offsets
    Ltot = int(grp_off[-1])

    idx_arrs, code_arrs = [], []
    for c in range(cfg.NC):
        key_s, rel_s, code_s, counts = per_core[c]
        cnt_off = np.concatenate([[0], np.cumsum(counts)])
        # destination position for each (sorted) edge
        pos_in_grp = np.arange(len(key_s), dtype=np.int64) - cnt_off[key_s]
        destp = grp_off[key_s] + pos_in_grp
        idx_flat = np.zeros(Ltot, np.int16)
        code_flat = np.full(Ltot, 255.0, np.float32)
        idx_flat[destp] = rel_s.astype(np.int16)
        code_flat[destp] = code_s
        # wrap idx into 16 partitions, replicate across the 8 Q7 stripes
        idx16 = idx_flat.reshape(-1, 16).T                      # [16, L/16]
        idx_arrs.append(np.ascontiguousarray(np.tile(idx16, (8, 1))))
        code_arrs.append(np.ascontiguousarray(code_flat.reshape(-1, 128).T).astype(BF16))

    # per-core norm tiles [128, BLK]
    nd_t, ns_t = [], []
    for c in range(cfg.NC):
        pad = np.zeros(cfg.PS, np.float32)
        pad[: cfg.SH] = nd[c * cfg.SH : (c + 1) * cfg.SH]
        nd_t.append(np.ascontiguousarray(pad.reshape(cfg.BLK, 128).T))
        pad2 = np.zeros(cfg.PS, np.float32)
        pad2[: cfg.SH] = ns[c * cfg.SH : (c + 1) * cfg.SH]
        ns_t.append(np.ascontiguousarray(pad2.reshape(cfg.BLK, 128).T))

    iota = np.ascontiguousarray(
        np.broadcast_to(
            np.tile(np.arange(128, dtype=np.float32), 4)[None, :], (128, 512)
        )
    ).astype(BF16)
    ident = np.eye(128, dtype=np.float32).astype(BF16)

    # bias terms (all-zero in this problem's spec; support nonzero anyway)
    bias = {}
    bias["b0row"] = np.asarray(b0, np.float32)
    bias["b1row"] = np.asarray(b1, np.float32)
    bias["b2row"] = np.asarray(b2, np.float32)
    bias["bw0row"] = (np.asarray(b_lin, np.float32) @ W0).astype(np.float32)
    if np.any(bias["bw0row"]):
        cvec = np.bincount(dst, weights=ns[src], minlength=N).astype(np.float32) * nd
        cn_t = []
        for c in range(cfg.NC):
            pad = np.zeros(cfg.PS, np.float32)
            pad[: cfg.SH] = cvec[c * cfg.SH : (c + 1) * cfg.SH]
            cn_t.append(np.ascontiguousarray(pad.reshape(cfg.BLK, 128).T))
        bias["cn_t"] = cn_t

    return dict(
        xt=xt,
        wc=wtile(wc, D), w1=wtile(np.asarray(W1, np.float32), D),
        w2=wtile(np.asarray(W2, np.float32), C),
        idx=idx_arrs, codes=code_arrs, nd=nd_t, ns=ns_t,
        iota=iota, ident=ident, T_bw=T_bw, Ltot=Ltot, bias=bias,
    )


def _build(cfg, T_bw, bias_en):
    """bias_en: dict of bools {b0, b1, b2, blin}."""
    D, C = cfg.D, cfg.C
    T_b = T_bw.sum(axis=1)                      # tiles per block
    tile_off = np.concatenate([[0], np.cumsum(T_b)])     # codes col offset per block
    grp_off = np.concatenate([[0], np.cumsum(T_bw.reshape(-1) * 128)])  # idx elem offsets
    Ltot = int(grp_off[-1])
    Ltiles = int(tile_off[-1])

    # per-chunk slices
    ch_tile0 = [int(tile_off[ch * cfg.CHB]) for ch in range(cfg.NCH + 1)]
    ch_idx0 = [int(grp_off[ch * cfg.CHB * cfg.NW]) for ch in range(cfg.NCH + 1)]
    max_ch_tiles = max(ch_tile0[i + 1] - ch_tile0[i] for i in range(cfg.NCH))
    max_ch_idx = max(ch_idx0[i + 1] - ch_idx0[i] for i in range(cfg.NCH))
    max_T_b = int(T_b.max())

    nc = bacc.Bacc("TRN2", num_devices=cfg.NC, num_swdge_queues=4)
    f32, bf16, i16 = mybir.dt.float32, mybir.dt.bfloat16, mybir.dt.int16
    f8 = mybir.dt.float8e4

    xt = nc.declare_dram_parameter("xt", [cfg.RT, D], bf16, isOutput=False)
    idxs = nc.declare_dram_parameter("idxs", [128, Ltot // 16], i16, isOutput=False)
    codes = nc.declare_dram_parameter("codes", [128, Ltiles], bf16, isOutput=False)
    wc = nc.declare_dram_parameter("wc", [128, D // 128, D], bf16, isOutput=False)
    w1 = nc.declare_dram_parameter("w1", [128, D // 128, D], bf16, isOutput=False)
    w2 = nc.declare_dram_parameter("w2", [128, D // 128, C], bf16, isOutput=False)
    ndp = nc.declare_dram_parameter("nd", [128, cfg.BLK], f32, isOutput=False)
    nsp = nc.declare_dram_parameter("ns", [128, cfg.BLK], f32, isOutput=False)
    iota = nc.declare_dram_parameter("iota", [128, 4, 128], bf16, isOutput=False)
    ident = nc.declare_dram_parameter("ident", [128, 128], bf16, isOutput=False)
    bias_p = {}
    if bias_en["b0"]:
        bias_p["b0row"] = nc.declare_dram_parameter("b0row", [128, D], f32, isOutput=False)
    if bias_en["b1"]:
        bias_p["b1row"] = nc.declare_dram_parameter("b1row", [128, D], f32, isOutput=False)
    if bias_en["b2"]:
        bias_p["b2row"] = nc.declare_dram_parameter("b2row", [128, C], f32, isOutput=False)
    if bias_en["blin"]:
        bias_p["bw0row"] = nc.declare_dram_parameter("bw0row", [128, D], f32, isOutput=False)
        bias_p["cn"] = nc.declare_dram_parameter("cn", [128, cfg.BLK], f32, isOutput=False)
    out = nc.declare_dram_parameter("out", [cfg.PS, C], f32, isOutput=True)

    h1s = [nc.dram_tensor(f"h1s_{i}", [cfg.CHR, D], f8) for i in range(cfg.NCH)]
    h2s = [nc.dram_tensor(f"h2s_{i}", [cfg.CHR, D], f8) for i in range(cfg.NCH)]
    h1f = nc.dram_tensor("h1f", [cfg.RT, D], f8, addr_space="Shared")
    h2f = nc.dram_tensor("h2f", [cfg.RT, D], f8, addr_space="Shared")

    cores = list(range(cfg.NC))

    with tile.TileContext(nc) as tc:
        with tc.tile_pool(name="const", bufs=1) as cp, \
             tc.tile_pool(name="chk", bufs=2) as kp, \
             tc.tile_pool(name="stag", bufs=2) as sp, \
             tc.tile_pool(name="work", bufs=3) as wp, \
             tc.tile_pool(name="psA", bufs=2, space="PSUM") as psA, \
             tc.tile_pool(name="psT", bufs=2, space="PSUM") as psT, \
             tc.tile_pool(name="psD", bufs=2, space="PSUM") as psD:

            iota_sb = cp.tile([128, 4, 128], bf16)
            nc.sync.dma_start(out=iota_sb[:], in_=iota[:])
            ident_sb = cp.tile([128, 128], bf16)
            nc.sync.dma_start(out=ident_sb[:], in_=ident[:])
            nd_sb = cp.tile([128, cfg.BLK], f32)
            nc.sync.dma_start(out=nd_sb[:], in_=ndp[:])
            ns_sb = cp.tile([128, cfg.BLK], f32)
            nc.sync.dma_start(out=ns_sb[:], in_=nsp[:])
            w_sb = {}
            for name, par, no in (("wc", wc, D), ("w1", w1, D), ("w2", w2, C)):
                t = cp.tile([128, D // 128, no], bf16, tag=f"w_{name}")
                nc.sync.dma_start(out=t[:], in_=par[:])
                w_sb[name] = t
            bias_sb = {}
            for nm in ("b0row", "b1row", "b2row", "bw0row"):
                if nm in bias_p:
                    no = C if nm == "b2row" else D
                    t = cp.tile([128, no], f32, tag=f"bias_{nm}")
                    nc.sync.dma_start(out=t[:], in_=bias_p[nm][:])
                    bias_sb[nm] = t
            if "cn" in bias_p:
                t = cp.tile([128, cfg.BLK], f32)
                nc.sync.dma_start(out=t[:], in_=bias_p["cn"][:])
                bias_sb["cn"] = t

            self_qn = [0]
            layers = (
                ("wc", xt, h1s, h1f, True, "b0row"),
                ("w1", h1f, h2s, h2f, True, "b1row"),
                ("w2", h2f, None, None, False, "b2row"),
            )
            for li, (wname, srct, hshard, hfull, relu_ns, brow) in enumerate(layers):
                NO = D if li < 2 else C
                gdt = bf16 if li == 0 else f8
                wt = w_sb[wname]
                for ch in range(cfg.NCH):
                    nt = ch_tile0[ch + 1] - ch_tile0[ch]
                    ni = ch_idx0[ch + 1] - ch_idx0[ch]
                    code_ch = kp.tile([128, max_ch_tiles], bf16, tag="code_ch")
                    nc.sync.dma_start(
                        out=code_ch[:, :nt], in_=codes[:, ch_tile0[ch] : ch_tile0[ch + 1]]
                    )
                    idx_ch = kp.tile([128, max_ch_idx // 16], i16, tag="idx_ch")
                    nc.sync.dma_start(
                        out=idx_ch[:, : ni // 16],
                        in_=idxs[:, ch_idx0[ch] // 16 : ch_idx0[ch + 1] // 16],
                    )
                    for bl in range(cfg.CHB):
                        b = ch * cfg.CHB + bl
                        tb = int(T_b[b])
                        stag = sp.tile([128, max_T_b, D], gdt, tag="stag")
                        o = 0
                        for w in range(cfg.NW):
                            tw = int(T_bw[b, w])
                            if tw == 0:
                                continue
                            i0 = (int(grp_off[b * cfg.NW + w]) - ch_idx0[ch]) // 16
                            wlo = w * cfg.WIN
                            whi = min(wlo + cfg.WIN, cfg.RT)
                            # dma_gather caps at 1024 idxs (128 descs x 8 Q7
                            # cores); split and round-robin the 4 SWDGE queues
                            for t0 in range(0, tw, 8):
                                tn = min(8, tw - t0)
                                nc.gpsimd.dma_gather(
                                    out_ap=stag[:, o + t0 : o + t0 + tn, :],
                                    in_ap=srct[wlo:whi, :],
                                    idxs_ap=idx_ch[:, i0 + t0 * 8 : i0 + (t0 + tn) * 8],
                                    num_idxs=tn * 128,
                                    num_idxs_reg=tn * 128,
                                    elem_size=D,
                                    queue_num=self_qn[0] % 4,
                                )
                                self_qn[0] += 1
                            o += tw
                        acc = psA.tile([128, D], f32, space="PSUM", tag="acc")
                        cb = int(tile_off[b]) - ch_tile0[ch]
                        for g in range(0, tb, 4):
                            gn = min(4, tb - g)
                            oh = wp.tile([128, 4, 128], gdt, tag="oh")
                            nc.vector.tensor_tensor(
                                out=oh[:, :gn, :],
                                in0=code_ch[:, cb + g : cb + g + gn].to_broadcast(
                                    [128, gn, 128]
                                ),
                                in1=iota_sb[:, :gn, :],
                                op=mybir.AluOpType.is_equal,
                            )
                            for k in range(gn):
                                ti = g + k
                                nc.tensor.matmul(
                                    out=acc[:], lhsT=oh[:, k, :], rhs=stag[:, ti, :],
                                    start=(ti == 0), stop=(ti == tb - 1),
                                )
                        # scale by dst-degree norm, cast bf16
                        mn = wp.tile([128, D], bf16, tag="mn")
                        nc.vector.tensor_scalar_mul(mn[:], acc[:], nd_sb[:, b : b + 1])
                        # transpose mn -> [f_in, nodes] chunks
                        pT = psT.tile([128, D], bf16, space="PSUM", tag="pT")
                        for j in range(D // 128):
                            nc.tensor.transpose(
                                out=pT[:, j * 128 : (j + 1) * 128],
                                in_=mn[:, j * 128 : (j + 1) * 128],
                                identity=ident_sb[:],
                            )
                        lhsT = wp.tile([128, D], bf16, tag="lhsT")
                        nc.vector.tensor_copy(out=lhsT[:], in_=pT[:])
                        pd = psD.tile([128, NO], f32, space="PSUM", tag="pd")
                        for j in range(D // 128):
                            nc.tensor.matmul(
                                out=pd[:], lhsT=lhsT[:, j * 128 : (j + 1) * 128],
                                rhs=wt[:, j, :], start=(j == 0), stop=(j == D // 128 - 1),
                            )
                        # epilogue
                        pre = pd
                        if li == 0 and "bw0row" in bias_sb:
                            tmp = wp.tile([128, D], f32, tag="btmp")
                            nc.vector.tensor_scalar_mul(
                                tmp[:], bias_sb["bw0row"][:], bias_sb["cn"][:, b : b + 1]
                            )
                            nc.vector.tensor_add(pd[:], pd[:], tmp[:])
                        if brow in bias_sb:
                            nc.vector.tensor_add(pd[:], pd[:], bias_sb[brow][:])
                        if relu_ns:
                            ht = wp.tile([128, D], f8, tag="ht")
                            nc.scalar.activation(
                                ht[:], pre[:],
                                mybir.ActivationFunctionType.Relu,
                                scale=ns_sb[:, b : b + 1],
                            )
                            nc.sync.dma_start(
                                out=hshard[ch][bl * 128 : (bl + 1) * 128, :], in_=ht[:]
                            )
                        else:
                            ot = wp.tile([128, C], f32, tag="ot")
                            nc.vector.tensor_copy(out=ot[:], in_=pd[:])
                            nc.sync.dma_start(
                                out=out[b * 128 : (b + 1) * 128, :], in_=ot[:]
                            )
                    if li < 2:
                        nc.gpsimd.collective_compute(
                            "AllGather",
                            mybir.AluOpType.bypass,
                            ins=[hshard[ch][:]],
                            outs=[
                                hfull[
                                    ch * cfg.NC * cfg.CHR : (ch + 1) * cfg.NC * cfg.CHR, :
                                ]
                            ],
                            replica_groups=[cores],
                        )
    nc.compile()
    return nc


_CACHE = {}


def _run(cfg, inputs, trace=False):
    pre = _preprocess(cfg, **inputs)
    bias_en = {
        "b0": bool(np.any(pre["bias"]["b0row"])),
        "b1": bool(np.any(pre["bias"]["b1row"])),
        "b2": bool(np.any(pre["bias"]["b2row"])),
        "blin": bool(np.any(pre["bias"]["bw0row"])),
    }
    key = (id(cfg), tuple(pre["T_bw"].reshape(-1)), tuple(sorted(bias_en.items())))
    if key not in _CACHE:
        _CACHE[key] = _build(cfg, pre["T_bw"], bias_en)
    nc = _CACHE[key]

    in_maps = []
    for c in range(cfg.NC):
        m = dict(
            xt=pre["xt"],
            idxs=pre["idx"][c], codes=pre["codes"][c],
            wc=pre["wc"], w1=pre["w1"], w2=pre["w2"],
            nd=pre["nd"][c], ns=pre["ns"][c],
            iota=pre["iota"], ident=pre["ident"],
        )
        rep = lambda v: np.ascontiguousarray(np.broadcast_to(v[None, :], (128, v.shape[0])))
        if bias_en["b0"]:
            m["b0row"] = rep(pre["bias"]["b0row"])
        if bias_en["b1"]:
            m["b1row"] = rep(pre["bias"]["b1row"])
        if bias_en["b2"]:
            m["b2row"] = rep(pre["bias"]["b2row"])
        if bias_en["blin"]:
            m["bw0row"] = rep(pre["bias"]["bw0row"])
            m["cn"] = pre["bias"]["cn_t"][c]
        in_maps.append(m)

    r = run_bass_kernel_spmd(nc, in_maps, list(range(cfg.NC)), trace=trace)
    outs = [np.asarray(r.results[c]["out"])[: cfg.SH] for c in range(cfg.NC)]
    full = np.concatenate(outs, axis=0)[: cfg.N]
    return full, r


def kernel(**inputs):
    inputs = {k: np.asarray(v) for k, v in inputs.items()}
    out, _ = _run(CFG, inputs)
    return out


# revision 9
# speedup vs baseline: 1.4528x; 1.0926x over previous
"""GCN (3-layer graph conv) on 8 Trainium2 NeuronCores.

Strategy (graph/data parallel, dst-sharded):
- Nodes sharded across 8 cores (12500 each, padded to 12544 = 98 blocks of 128).
- Aggregation m[v] = sum_{(u,v) in E} h[u]: per 128-dst block, dma_gather
  fetches the src rows (bf16, 1KB each) in 128-edge tiles; segment-sum is a
  TensorE matmul with a DVE-built one-hot [128 edges x 128 dst] as the
  stationary operand, accumulating into one PSUM bank.
- Dense h@W: PE-transpose the scaled block then 4 K-block matmuls against the
  resident weight; epilogue fuses ReLU and the next layer's src-degree norm
  via ScalarE activation(scale=...).
- W_lin is folded into layer 0 (A(ns*X) @ (W_lin@W0) == A(ns*(X W_lin)) @ W0),
  so only 3 aggregation+dense rounds run on device.
- Inter-layer exchange: full h is rebuilt on every core by 7 chunked
  AllGathers per layer (14 blocks each) so they can overlap block compute.
- Gather tables use a [chunk][core][row] layout so each AllGather writes a
  contiguous slice; gather indices are window-relative int16 (4 windows of
  32768 rows), replicated across the 8 Q7 descriptor-generator stripes.
"""

import math
import numpy as np
import ml_dtypes

import concourse.bass as bass
import concourse.tile as tile
from concourse import bacc, mybir
from concourse.bass_utils import run_bass_kernel_spmd

BF16 = ml_dtypes.bfloat16


class Cfg:
    def __init__(self, N, E, D, C, win, blocks_per_chunk):
        self.N, self.E, self.D, self.C = N, E, D, C
        self.NC = 8
        assert N % self.NC == 0
        self.SH = N // self.NC                       # nodes per core
        self.BLK = math.ceil(self.SH / 128)          # 128-row blocks per core
        self.CHB = blocks_per_chunk                  # blocks per collective chunk
        assert self.BLK % self.CHB == 0
        self.NCH = self.BLK // self.CHB              # collective chunks
        self.PS = self.BLK * 128                     # padded shard rows
        self.CHR = self.CHB * 128                    # shard rows per chunk
        self.RT = self.NC * self.PS                  # full padded table rows
        self.WIN = win                               # gather window rows
        self.NW = math.ceil(self.RT / win)
        # per-chunk full-table tensors require window == chunk region
        assert win == self.NC * self.CHR, (win, self.NC * self.CHR)
        assert win <= 32768


CFG = Cfg(N=100000, E=3200000, D=512, C=40, win=14336, blocks_per_chunk=14)


def _rows_of(v, cfg):
    """Padded table row of node v under the [chunk][core][row] layout."""
    c = v // cfg.SH
    l = v % cfg.SH
    k = l // cfg.CHR
    r = l % cfg.CHR
    return (k * cfg.NC * cfg.CHR) + c * cfg.CHR + r


def _preprocess(cfg, features, src, dst, W_lin, b_lin, W0, b0, W1, b1, W2, b2):
    N, E, D, C = cfg.N, cfg.E, cfg.D, cfg.C
    deg_out = np.bincount(src, minlength=N).astype(np.float32)
    deg_in = np.bincount(dst, minlength=N).astype(np.float32)
    ns = np.maximum(deg_out, 1.0) ** -0.5
    nd = np.maximum(deg_in, 1.0) ** -0.5

    # gather table for layer 0: ns * X, padded layout, bf16
    rows = _rows_of(np.arange(N, dtype=np.int64), cfg)
    xt = np.zeros((cfg.RT, D), dtype=BF16)
    xt[rows] = (features * ns[:, None]).astype(BF16)

    # folded weights
    wc = (W_lin @ W0).astype(np.float32)

    def wtile(W, no):
        # [D, no] -> [128, D//128, no]
        return np.ascontiguousarray(
            W.reshape(D // 128, 128, no).transpose(1, 0, 2)
        ).astype(BF16)

    srow = _rows_of(src.astype(np.int64), cfg)
    swin = (srow // cfg.WIN).astype(np.int64)
    srel = (srow % cfg.WIN).astype(np.int64)

    # per-core edge grouping by (dst block, src window)
    per_core = []
    counts_all = np.zeros((cfg.NC, cfg.BLK * cfg.NW), dtype=np.int64)
    for c in range(cfg.NC):
        sel = (dst >= c * cfg.SH) & (dst < (c + 1) * cfg.SH)
        l = dst[sel].astype(np.int64) - c * cfg.SH
        b = l // 128
        code = (l % 128).astype(np.float32)
        w = swin[sel]
        rel = srel[sel]
        key = b * cfg.NW + w
        order = np.argsort(key, kind="stable")
        key_s = key[order]
        counts = np.bincount(key_s, minlength=cfg.BLK * cfg.NW)
        counts_all[c] = counts
        per_core.append((key_s, rel[order], code[order], counts))

    # global tile counts per (block, window)
    T_bw = np.ceil(counts_all.max(axis=0) / 128.0).astype(np.int64).reshape(cfg.BLK, cfg.NW)
    grp_pad = (T_bw.reshape(-1) * 128)                  # padded edges per group
    grp_off = np.concatenate([[0], np.cumsum(grp_pad)]) # element offsets
    Ltot = int(grp_off[-1])

    idx_arrs, code_arrs = [], []
    for c in range(cfg.NC):
        key_s, rel_s, code_s, counts = per_core[c]
        cnt_off = np.concatenate([[0], np.cumsum(counts)])
        # destination position for each (sorted) edge
        pos_in_grp = np.arange(len(key_s), dtype=np.int64) - cnt_off[key_s]
        destp = grp_off[key_s] + pos_in_grp
        idx_flat = np.zeros(Ltot, np.int16)
        code_flat = np.full(Ltot, 255.0, np.float32)
        idx_flat[destp] = rel_s.astype(np.int16)
        code_flat[destp] = code_s
        # wrap idx into 16 partitions, replicate across the 8 Q7 stripes
        idx16 = idx_flat.reshape(-1, 16).T                      # [16, L/16]
        idx_arrs.append(np.ascontiguousarray(np.tile(idx16, (8, 1))))
        code_arrs.append(np.ascontiguousarray(code_flat.reshape(-1, 128).T).astype(BF16))

    # per-core norm tiles [128, BLK]
    nd_t, ns_t = [], []
    for c in range(cfg.NC):
        pad = np.zeros(cfg.PS, np.float32)
        pad[: cfg.SH] = nd[c * cfg.SH : (c + 1) * cfg.SH]
        nd_t.append(np.ascontiguousarray(pad.reshape(cfg.BLK, 128).T))
        pad2 = np.zeros(cfg.PS, np.float32)
        pad2[: cfg.SH] = ns[c * cfg.SH : (c + 1) * cfg.SH]
        ns_t.append(np.ascontiguousarray(pad2.reshape(cfg.BLK, 128).T))

    iota = np.ascontiguousarray(
        np.broadcast_to(
            np.tile(np.arange(128, dtype=np.float32), 4)[None, :], (128, 512)
        )
    ).astype(BF16)
    ident = np.eye(128, dtype=np.float32).astype(BF16)

    # bias terms (all-zero in this problem's spec; support nonzero anyway)
    bias = {}
    bias["b0row"] = np.asarray(b0, np.float32)
    bias["b1row"] = np.asarray(b1, np.float32)
    bias["b2row"] = np.asarray(b2, np.float32)
    bias["bw0row"] = (np.asarray(b_lin, np.float32) @ W0).astype(np.float32)
    if np.any(bias["bw0row"]):
        cvec = np.bincount(dst, weights=ns[src], minlength=N).astype(np.float32) * nd
        cn_t = []
        for c in range(cfg.NC):
            pad = np.zeros(cfg.PS, np.float32)
            pad[: cfg.SH] = cvec[c * cfg.SH : (c + 1) * cfg.SH]
            cn_t.append(np.ascontiguousarray(pad.reshape(cfg.BLK, 128).T))
        bias["cn_t"] = cn_t

    return dict(
        xt=xt,
        wc=wtile(wc, D), w1=wtile(np.asarray(W1, np.float32), D),
        w2=wtile(np.asarray(W2, np.float32), C),
        idx=idx_arrs, codes=code_arrs, nd=nd_t, ns=ns_t,
        iota=iota, ident=ident, T_bw=T_bw, Ltot=Ltot, bias=bias,
    )


def _build(cfg, T_bw, bias_en):
    """bias_en: dict of bools {b0, b1, b2, blin}."""
    D, C = cfg.D, cfg.C
    T_b = T_bw.sum(axis=1)                      # tiles per block
    tile_off = np.concatenate([[0], np.cumsum(T_b)])     # codes col offset per block
    grp_off = np.concatenate([[0], np.cumsum(T_bw.reshape(-1) * 128)])  # idx elem offsets
    Ltot = int(grp_off[-1])
    Ltiles = int(tile_off[-1])

    # per-chunk slices
    ch_tile0 = [int(tile_off[ch * cfg.CHB]) for ch in range(cfg.NCH + 1)]
    ch_idx0 = [int(grp_off[ch * cfg.CHB * cfg.NW]) for ch in range(cfg.NCH + 1)]
    max_ch_tiles = max(ch_tile0[i + 1] - ch_tile0[i] for i in range(cfg.NCH))
    max_ch_idx = max(ch_idx0[i + 1] - ch_idx0[i] for i in range(cfg.NCH))
    max_T_b = int(T_b.max())

    nc = bacc.Bacc("TRN2", num_devices=cfg.NC, num_swdge_queues=4)
    f32, bf16, i16 = mybir.dt.float32, mybir.dt.bfloat16, mybir.dt.int16
    f8 = mybir.dt.float8e4

    xt = nc.declare_dram_parameter("xt", [cfg.RT, D], bf16, isOutput=False)
    idxs = nc.declare_dram_parameter("idxs", [128, Ltot // 16], i16, isOutput=False)
    codes = nc.declare_dram_parameter("codes", [128, Ltiles], bf16, isOutput=False)
    wc = nc.declare_dram_parameter("wc", [128, D // 128, D], bf16, isOutput=False)
    w1 = nc.declare_dram_parameter("w1", [128, D // 128, D], bf16, isOutput=False)
    w2 = nc.declare_dram_parameter("w2", [128, D // 128, C], bf16, isOutput=False)
    ndp = nc.declare_dram_parameter("nd", [128, cfg.BLK], f32, isOutput=False)
    nsp = nc.declare_dram_parameter("ns", [128, cfg.BLK], f32, isOutput=False)
    iota = nc.declare_dram_parameter("iota", [128, 4, 128], bf16, isOutput=False)
    ident = nc.declare_dram_parameter("ident", [128, 128], bf16, isOutput=False)
    bias_p = {}
    if bias_en["b0"]:
        bias_p["b0row"] = nc.declare_dram_parameter("b0row", [128, D], f32, isOutput=False)
    if bias_en["b1"]:
        bias_p["b1row"] = nc.declare_dram_parameter("b1row", [128, D], f32, isOutput=False)
    if bias_en["b2"]:
        bias_p["b2row"] = nc.declare_dram_parameter("b2row", [128, C], f32, isOutput=False)
    if bias_en["blin"]:
        bias_p["bw0row"] = nc.declare_dram_parameter("bw0row", [128, D], f32, isOutput=False)
        bias_p["cn"] = nc.declare_dram_parameter("cn", [128, cfg.BLK], f32, isOutput=False)
    out = nc.declare_dram_parameter("out", [cfg.PS, C], f32, isOutput=True)

    h1s = [nc.dram_tensor(f"h1s_{i}", [cfg.CHR, D], f8) for i in range(cfg.NCH)]
    h2s = [nc.dram_tensor(f"h2s_{i}", [cfg.CHR, D], f8) for i in range(cfg.NCH)]
    CW = cfg.NC * cfg.CHR
    h1f = [nc.dram_tensor(f"h1f_{i}", [CW, D], f8, addr_space="Shared") for i in range(cfg.NCH)]
    h2f = [nc.dram_tensor(f"h2f_{i}", [CW, D], f8, addr_space="Shared") for i in range(cfg.NCH)]

    cores = list(range(cfg.NC))

    with tile.TileContext(nc) as tc:
        with tc.tile_pool(name="const", bufs=1) as cp, \
             tc.tile_pool(name="chk", bufs=2) as kp, \
             tc.tile_pool(name="stag", bufs=2) as sp, \
             tc.tile_pool(name="work", bufs=3) as wp, \
             tc.tile_pool(name="psA", bufs=2, space="PSUM") as psA, \
             tc.tile_pool(name="psT", bufs=2, space="PSUM") as psT, \
             tc.tile_pool(name="psD", bufs=2, space="PSUM") as psD:

            iota_sb = cp.tile([128, 4, 128], bf16)
            nc.sync.dma_start(out=iota_sb[:], in_=iota[:])
            ident_sb = cp.tile([128, 128], bf16)
            nc.sync.dma_start(out=ident_sb[:], in_=ident[:])
            nd_sb = cp.tile([128, cfg.BLK], f32)
            nc.sync.dma_start(out=nd_sb[:], in_=ndp[:])
            ns_sb = cp.tile([128, cfg.BLK], f32)
            nc.sync.dma_start(out=ns_sb[:], in_=nsp[:])
            w_sb = {}
            for name, par, no in (("wc", wc, D), ("w1", w1, D), ("w2", w2, C)):
                t = cp.tile([128, D // 128, no], bf16, tag=f"w_{name}")
                nc.sync.dma_start(out=t[:], in_=par[:])
                w_sb[name] = t
            bias_sb = {}
            for nm in ("b0row", "b1row", "b2row", "bw0row"):
                if nm in bias_p:
                    no = C if nm == "b2row" else D
                    t = cp.tile([128, no], f32, tag=f"bias_{nm}")
                    nc.sync.dma_start(out=t[:], in_=bias_p[nm][:])
                    bias_sb[nm] = t
            if "cn" in bias_p:
                t = cp.tile([128, cfg.BLK], f32)
                nc.sync.dma_start(out=t[:], in_=bias_p["cn"][:])
                bias_sb["cn"] = t

            self_qn = [0]
            layers = (
                ("wc", xt, h1s, h1f, True, "b0row"),
                ("w1", h1f, h2s, h2f, True, "b1row"),
                ("w2", h2f, None, None, False, "b2row"),
            )
            for li, (wname, srct, hshard, hfull, relu_ns, brow) in enumerate(layers):
                NO = D if li < 2 else C
                gdt = bf16 if li == 0 else f8
                wt = w_sb[wname]
                for ch in range(cfg.NCH):
                    nt = ch_tile0[ch + 1] - ch_tile0[ch]
                    ni = ch_idx0[ch + 1] - ch_idx0[ch]
                    code_ch = kp.tile([128, max_ch_tiles], bf16, tag="code_ch")
                    nc.sync.dma_start(
                        out=code_ch[:, :nt], in_=codes[:, ch_tile0[ch] : ch_tile0[ch + 1]]
                    )
                    idx_ch = kp.tile([128, max_ch_idx // 16], i16, tag="idx_ch")
                    nc.sync.dma_start(
                        out=idx_ch[:, : ni // 16],
                        in_=idxs[:, ch_idx0[ch] // 16 : ch_idx0[ch + 1] // 16],
                    )
                    for bl in range(cfg.CHB):
                        b = ch * cfg.CHB + bl
                        tb = int(T_b[b])
                        stag = sp.tile([128, max_T_b, D], gdt, tag="stag")
                        o = 0
                        for w in range(cfg.NW):
                            tw = int(T_bw[b, w])
                            if tw == 0:
                                continue
                            i0 = (int(grp_off[b * cfg.NW + w]) - ch_idx0[ch]) // 16
                            if isinstance(srct, list):
                                src_ap = srct[w][:]
                            else:
                                wlo = w * cfg.WIN
                                whi = min(wlo + cfg.WIN, cfg.RT)
                                src_ap = srct[wlo:whi, :]
                            # dma_gather caps at 1024 idxs (128 descs x 8 Q7
                            # cores); split and round-robin the 4 SWDGE queues
                            for t0 in range(0, tw, 8):
                                tn = min(8, tw - t0)
                                nc.gpsimd.dma_gather(
                                    out_ap=stag[:, o + t0 : o + t0 + tn, :],
                                    in_ap=src_ap,
                                    idxs_ap=idx_ch[:, i0 + t0 * 8 : i0 + (t0 + tn) * 8],
                                    num_idxs=tn * 128,
                                    num_idxs_reg=tn * 128,
                                    elem_size=D,
                                    queue_num=self_qn[0] % 4,
                                )
                                self_qn[0] += 1
                            o += tw
                        acc = psA.tile([128, D], f32, space="PSUM", tag="acc")
                        cb = int(tile_off[b]) - ch_tile0[ch]
                        for g in range(0, tb, 4):
                            gn = min(4, tb - g)
                            oh = wp.tile([128, 4, 128], gdt, tag="oh")
                            nc.vector.tensor_tensor(
                                out=oh[:, :gn, :],
                                in0=code_ch[:, cb + g : cb + g + gn].to_broadcast(
                                    [128, gn, 128]
                                ),
                                in1=iota_sb[:, :gn, :],
                                op=mybir.AluOpType.is_equal,
                            )
                            for k in range(gn):
                                ti = g + k
                                nc.tensor.matmul(
                                    out=acc[:], lhsT=oh[:, k, :], rhs=stag[:, ti, :],
                                    start=(ti == 0), stop=(ti == tb - 1),
                                )
                        # scale by dst-degree norm, cast bf16
                        mn = wp.tile([128, D], bf16, tag="mn")
                        nc.vector.tensor_scalar_mul(mn[:], acc[:], nd_sb[:, b : b + 1])
                        # transpose mn -> [f_in, nodes] chunks
                        pT = psT.tile([128, D], bf16, space="PSUM", tag="pT")
                        for j in range(D // 128):
                            nc.tensor.transpose(
                                out=pT[:, j * 128 : (j + 1) * 128],
                                in_=mn[:, j * 128 : (j + 1) * 128],
                                identity=ident_sb[:],
                            )
                        lhsT = wp.tile([128, D], bf16, tag="lhsT")
                        nc.vector.tensor_copy(out=lhsT[:], in_=pT[:])
                        pd = psD.tile([128, NO], f32, space="PSUM", tag="pd")
                        for j in range(D // 128):
                            nc.tensor.matmul(
                                out=pd[:], lhsT=lhsT[:, j * 128 : (j + 1) * 128],
                                rhs=wt[:, j, :], start=(j == 0), stop=(j == D // 128 - 1),
                            )
                        # epilogue
                        pre = pd
                        if li == 0 and "bw0row" in bias_sb:
                            tmp = wp.tile([128, D], f32, tag="btmp")
                            nc.vector.tensor_scalar_mul(
                                tmp[:], bias_sb["bw0row"][:], bias_sb["cn"][:, b : b + 1]
                            )
                            nc.vector.tensor_add(pd[:], pd[:], tmp[:])
                        if brow in bias_sb:
                            nc.vector.tensor_add(pd[:], pd[:], bias_sb[brow][:])
                        if relu_ns:
                            ht = wp.tile([128, D], f8, tag="ht")
                            nc.scalar.activation(
                                ht[:], pre[:],
                                mybir.ActivationFunctionType.Relu,
                                scale=ns_sb[:, b : b + 1],
                            )
                            nc.sync.dma_start(
                                out=hshard[ch][bl * 128 : (bl + 1) * 128, :], in_=ht[:]
                            )
                        else:
                            ot = wp.tile([128, C], f32, tag="ot")
                            nc.vector.tensor_copy(out=ot[:], in_=pd[:])
                            nc.sync.dma_start(
                                out=out[b * 128 : (b + 1) * 128, :], in_=ot[:]
                            )
                    if li < 2:
                        nc.gpsimd.collective_compute(
                            "AllGather",
                            mybir.AluOpType.bypass,
                            ins=[hshard[ch][:]],
                            outs=[hfull[ch][:]],
                            replica_groups=[cores],
                        )
    nc.compile()
    return nc


_CACHE = {}


def _run(cfg, inputs, trace=False):
    pre = _preprocess(cfg, **inputs)
    bias_en = {
        "b0": bool(np.any(pre["bias"]["b0row"])),
        "b1": bool(np.any(pre["bias"]["b1row"])),
        "b2": bool(np.any(pre["bias"]["b2row"])),
        "blin": bool(np.any(pre["bias"]["bw0row"])),
    }
    key = (id(cfg), tuple(pre["T_bw"].reshape(-1)), tuple(sorted(bias_en.items())))
    if key not in _CACHE:
        _CACHE[key] = _build(cfg, pre["T_bw"], bias_en)
    nc = _CACHE[key]

    in_maps = []
    for c in range(cfg.NC):
        m = dict(
            xt=pre["xt"],
            idxs=pre["idx"][c], codes=pre["codes"][c],
            wc=pre["wc"], w1=pre["w1"], w2=pre["w2"],
            nd=pre["nd"][c], ns=pre["ns"][c],
            iota=pre["iota"], ident=pre["ident"],
        )
        rep = lambda v: np.ascontiguousarray(np.broadcast_to(v[None, :], (128, v.shape[0])))
        if bias_en["b0"]:
            m["b0row"] = rep(pre["bias"]["b0row"])
        if bias_en["b1"]:
            m["b1row"] = rep(pre["bias"]["b1row"])
        if bias_en["b2"]:
            m["b2row"] = rep(pre["bias"]["b2row"])
        if bias_en["blin"]:
            m["bw0row"] = rep(pre["bias"]["bw0row"])
            m["cn"] = pre["bias"]["cn_t"][c]
        in_maps.append(m)

    r = run_bass_kernel_spmd(nc, in_maps, list(range(cfg.NC)), trace=trace)
    outs = [np.asarray(r.results[c]["out"])[: cfg.SH] for c in range(cfg.NC)]
    full = np.concatenate(outs, axis=0)[: cfg.N]
    return full, r


def kernel(**inputs):
    inputs = {k: np.asarray(v) for k, v in inputs.items()}
    out, _ = _run(CFG, inputs)
    return out


# revision 13
# speedup vs baseline: 1.5343x; 1.0561x over previous
"""GCN (3-layer graph conv) on 8 Trainium2 NeuronCores.

Strategy (graph/data parallel, dst-sharded):
- Nodes sharded across 8 cores (12500 each, padded to 12544 = 98 blocks of 128).
- Aggregation m[v] = sum_{(u,v) in E} h[u]: per 128-dst block, dma_gather
  fetches the src rows (bf16, 1KB each) in 128-edge tiles; segment-sum is a
  TensorE matmul with a DVE-built one-hot [128 edges x 128 dst] as the
  stationary operand, accumulating into one PSUM bank.
- Dense h@W: PE-transpose the scaled block then 4 K-block matmuls against the
  resident weight; epilogue fuses ReLU and the next layer's src-degree norm
  via ScalarE activation(scale=...).
- W_lin is folded into layer 0 (A(ns*X) @ (W_lin@W0) == A(ns*(X W_lin)) @ W0),
  so only 3 aggregation+dense rounds run on device.
- Inter-layer exchange: full h is rebuilt on every core by 7 chunked
  AllGathers per layer (14 blocks each) so they can overlap block compute.
- Gather tables use a [chunk][core][row] layout so each AllGather writes a
  contiguous slice; gather indices are window-relative int16 (4 windows of
  32768 rows), replicated across the 8 Q7 descriptor-generator stripes.
"""

import math
import numpy as np
import ml_dtypes

import concourse.bass as bass
import concourse.tile as tile
from concourse import bacc, mybir
from concourse.bass_utils import run_bass_kernel_spmd

BF16 = ml_dtypes.bfloat16


class Cfg:
    def __init__(self, N, E, D, C, win, blocks_per_chunk):
        self.N, self.E, self.D, self.C = N, E, D, C
        self.NC = 8
        assert N % self.NC == 0
        self.SH = N // self.NC                       # nodes per core
        self.BLK = math.ceil(self.SH / 128)          # 128-row blocks per core
        self.CHB = blocks_per_chunk                  # blocks per collective chunk
        assert self.BLK % self.CHB == 0
        self.NCH = self.BLK // self.CHB              # collective chunks
        self.PS = self.BLK * 128                     # padded shard rows
        self.CHR = self.CHB * 128                    # shard rows per chunk
        self.RT = self.NC * self.PS                  # full padded table rows
        self.WIN = win                               # gather window rows
        self.NW = math.ceil(self.RT / win)
        # per-chunk full-table tensors require window == chunk region
        assert win == self.NC * self.CHR, (win, self.NC * self.CHR)
        assert win <= 32768


CFG = Cfg(N=100000, E=3200000, D=512, C=40, win=14336, blocks_per_chunk=14)


def _rows_of(v, cfg):
    """Padded table row of node v under the [chunk][core][row] layout."""
    c = v // cfg.SH
    l = v % cfg.SH
    k = l // cfg.CHR
    r = l % cfg.CHR
    return (k * cfg.NC * cfg.CHR) + c * cfg.CHR + r


def _preprocess(cfg, features, src, dst, W_lin, b_lin, W0, b0, W1, b1, W2, b2):
    N, E, D, C = cfg.N, cfg.E, cfg.D, cfg.C
    deg_out = np.bincount(src, minlength=N).astype(np.float32)
    deg_in = np.bincount(dst, minlength=N).astype(np.float32)
    ns = np.maximum(deg_out, 1.0) ** -0.5
    nd = np.maximum(deg_in, 1.0) ** -0.5

    # gather table for layer 0: ns * X, padded layout, bf16
    rows = _rows_of(np.arange(N, dtype=np.int64), cfg)
    xt = np.zeros((cfg.RT, D), dtype=BF16)
    xt[rows] = (features * ns[:, None]).astype(BF16)

    # folded weights
    wc = (W_lin @ W0).astype(np.float32)

    def wtile(W, no):
        # [D, no] -> [128, D//128, no]
        return np.ascontiguousarray(
            W.reshape(D // 128, 128, no).transpose(1, 0, 2)
        ).astype(BF16)

    srow = _rows_of(src.astype(np.int64), cfg)
    swin = (srow // cfg.WIN).astype(np.int64)
    srel = (srow % cfg.WIN).astype(np.int64)

    # per-core edge grouping by (dst block, src window)
    per_core = []
    counts_all = np.zeros((cfg.NC, cfg.BLK * cfg.NW), dtype=np.int64)
    for c in range(cfg.NC):
        sel = (dst >= c * cfg.SH) & (dst < (c + 1) * cfg.SH)
        l = dst[sel].astype(np.int64) - c * cfg.SH
        b = l // 128
        code = (l % 128).astype(np.float32)
        w = swin[sel]
        rel = srel[sel]
        key = b * cfg.NW + w
        order = np.argsort(key, kind="stable")
        key_s = key[order]
        counts = np.bincount(key_s, minlength=cfg.BLK * cfg.NW)
        counts_all[c] = counts
        per_core.append((key_s, rel[order], code[order], counts))

    # global tile counts per (block, window)
    Vmax = counts_all.max(axis=0)                       # valid (descriptor) count
    T_bw = np.ceil(Vmax / 128.0).astype(np.int64).reshape(cfg.BLK, cfg.NW)
    grp_pad = (T_bw.reshape(-1) * 128)                  # padded edges per group
    grp_off = np.concatenate([[0], np.cumsum(grp_pad)]) # element offsets
    Ltot = int(grp_off[-1])

    idx_arrs, code_arrs = [], []
    for c in range(cfg.NC):
        key_s, rel_s, code_s, counts = per_core[c]
        cnt_off = np.concatenate([[0], np.cumsum(counts)])
        # destination position for each (sorted) edge
        pos_in_grp = np.arange(len(key_s), dtype=np.int64) - cnt_off[key_s]
        destp = grp_off[key_s] + pos_in_grp
        idx_flat = np.full(Ltot, -1, np.int16)
        code_flat = np.full(Ltot, 255.0, np.float32)
        idx_flat[destp] = rel_s.astype(np.int16)
        code_flat[destp] = code_s
        # idx-0 filler descriptors up to the uniform valid count V; -1 beyond
        for g in range(cfg.BLK * cfg.NW):
            lo = grp_off[g] + counts[g]
            hi = grp_off[g] + Vmax[g]
            idx_flat[lo:hi] = 0
        # wrap idx into 16 partitions, replicate across the 8 Q7 stripes
        idx16 = idx_flat.reshape(-1, 16).T                      # [16, L/16]
        idx_arrs.append(np.ascontiguousarray(np.tile(idx16, (8, 1))))
        code_arrs.append(np.ascontiguousarray(code_flat.reshape(-1, 128).T).astype(BF16))

    # per-core norm tiles [128, BLK]
    nd_t, ns_t = [], []
    for c in range(cfg.NC):
        pad = np.zeros(cfg.PS, np.float32)
        pad[: cfg.SH] = nd[c * cfg.SH : (c + 1) * cfg.SH]
        nd_t.append(np.ascontiguousarray(pad.reshape(cfg.BLK, 128).T))
        pad2 = np.zeros(cfg.PS, np.float32)
        pad2[: cfg.SH] = ns[c * cfg.SH : (c + 1) * cfg.SH]
        ns_t.append(np.ascontiguousarray(pad2.reshape(cfg.BLK, 128).T))

    iota = np.ascontiguousarray(
        np.broadcast_to(
            np.tile(np.arange(128, dtype=np.float32), 4)[None, :], (128, 512)
        )
    ).astype(BF16)
    ident = np.eye(128, dtype=np.float32).astype(BF16)

    # bias terms (all-zero in this problem's spec; support nonzero anyway)
    bias = {}
    bias["b0row"] = np.asarray(b0, np.float32)
    bias["b1row"] = np.asarray(b1, np.float32)
    bias["b2row"] = np.asarray(b2, np.float32)
    bias["bw0row"] = (np.asarray(b_lin, np.float32) @ W0).astype(np.float32)
    if np.any(bias["bw0row"]):
        cvec = np.bincount(dst, weights=ns[src], minlength=N).astype(np.float32) * nd
        cn_t = []
        for c in range(cfg.NC):
            pad = np.zeros(cfg.PS, np.float32)
            pad[: cfg.SH] = cvec[c * cfg.SH : (c + 1) * cfg.SH]
            cn_t.append(np.ascontiguousarray(pad.reshape(cfg.BLK, 128).T))
        bias["cn_t"] = cn_t

    return dict(
        xt=xt,
        wc=wtile(wc, D), w1=wtile(np.asarray(W1, np.float32), D),
        w2=wtile(np.asarray(W2, np.float32), C),
        idx=idx_arrs, codes=code_arrs, nd=nd_t, ns=ns_t,
        iota=iota, ident=ident, T_bw=T_bw, Vmax=Vmax, Ltot=Ltot, bias=bias,
    )


def _build(cfg, T_bw, Vmax, bias_en):
    """bias_en: dict of bools {b0, b1, b2, blin}."""
    D, C = cfg.D, cfg.C
    T_b = T_bw.sum(axis=1)                      # tiles per block
    tile_off = np.concatenate([[0], np.cumsum(T_b)])     # codes col offset per block
    grp_off = np.concatenate([[0], np.cumsum(T_bw.reshape(-1) * 128)])  # idx elem offsets
    Ltot = int(grp_off[-1])
    Ltiles = int(tile_off[-1])

    # per-chunk slices
    ch_tile0 = [int(tile_off[ch * cfg.CHB]) for ch in range(cfg.NCH + 1)]
    ch_idx0 = [int(grp_off[ch * cfg.CHB * cfg.NW]) for ch in range(cfg.NCH + 1)]
    max_ch_tiles = max(ch_tile0[i + 1] - ch_tile0[i] for i in range(cfg.NCH))
    max_ch_idx = max(ch_idx0[i + 1] - ch_idx0[i] for i in range(cfg.NCH))
    max_T_b = int(T_b.max())

    nc = bacc.Bacc("TRN2", num_devices=cfg.NC, num_swdge_queues=4)
    f32, bf16, i16 = mybir.dt.float32, mybir.dt.bfloat16, mybir.dt.int16
    f8 = mybir.dt.float8e4

    xt = nc.declare_dram_parameter("xt", [cfg.RT, D], bf16, isOutput=False)
    idxs = nc.declare_dram_parameter("idxs", [128, Ltot // 16], i16, isOutput=False)
    codes = nc.declare_dram_parameter("codes", [128, Ltiles], bf16, isOutput=False)
    wc = nc.declare_dram_parameter("wc", [128, D // 128, D], bf16, isOutput=False)
    w1 = nc.declare_dram_parameter("w1", [128, D // 128, D], bf16, isOutput=False)
    w2 = nc.declare_dram_parameter("w2", [128, D // 128, C], bf16, isOutput=False)
    ndp = nc.declare_dram_parameter("nd", [128, cfg.BLK], f32, isOutput=False)
    nsp = nc.declare_dram_parameter("ns", [128, cfg.BLK], f32, isOutput=False)
    iota = nc.declare_dram_parameter("iota", [128, 4, 128], bf16, isOutput=False)
    ident = nc.declare_dram_parameter("ident", [128, 128], bf16, isOutput=False)
    bias_p = {}
    if bias_en["b0"]:
        bias_p["b0row"] = nc.declare_dram_parameter("b0row", [128, D], f32, isOutput=False)
    if bias_en["b1"]:
        bias_p["b1row"] = nc.declare_dram_parameter("b1row", [128, D], f32, isOutput=False)
    if bias_en["b2"]:
        bias_p["b2row"] = nc.declare_dram_parameter("b2row", [128, C], f32, isOutput=False)
    if bias_en["blin"]:
        bias_p["bw0row"] = nc.declare_dram_parameter("bw0row", [128, D], f32, isOutput=False)
        bias_p["cn"] = nc.declare_dram_parameter("cn", [128, cfg.BLK], f32, isOutput=False)
    out = nc.declare_dram_parameter("out", [cfg.PS, C], f32, isOutput=True)

    h1s = [nc.dram_tensor(f"h1s_{i}", [cfg.CHR, D], f8) for i in range(cfg.NCH)]
    h2s = [nc.dram_tensor(f"h2s_{i}", [cfg.CHR, D], f8) for i in range(cfg.NCH)]
    CW = cfg.NC * cfg.CHR
    h1f = [nc.dram_tensor(f"h1f_{i}", [CW, D], f8, addr_space="Shared") for i in range(cfg.NCH)]
    h2f = [nc.dram_tensor(f"h2f_{i}", [CW, D], f8, addr_space="Shared") for i in range(cfg.NCH)]

    cores = list(range(cfg.NC))

    with tile.TileContext(nc) as tc:
        with tc.tile_pool(name="const", bufs=1) as cp, \
             tc.tile_pool(name="chk", bufs=2) as kp, \
             tc.tile_pool(name="stag", bufs=2) as sp, \
             tc.tile_pool(name="work", bufs=3) as wp, \
             tc.tile_pool(name="psA", bufs=2, space="PSUM") as psA, \
             tc.tile_pool(name="psT", bufs=2, space="PSUM") as psT, \
             tc.tile_pool(name="psD", bufs=2, space="PSUM") as psD:

            iota_sb = cp.tile([128, 4, 128], bf16)
            nc.sync.dma_start(out=iota_sb[:], in_=iota[:])
            ident_sb = cp.tile([128, 128], bf16)
            nc.sync.dma_start(out=ident_sb[:], in_=ident[:])
            nd_sb = cp.tile([128, cfg.BLK], f32)
            nc.sync.dma_start(out=nd_sb[:], in_=ndp[:])
            ns_sb = cp.tile([128, cfg.BLK], f32)
            nc.sync.dma_start(out=ns_sb[:], in_=nsp[:])
            w_sb = {}
            for name, par, no in (("wc", wc, D), ("w1", w1, D), ("w2", w2, C)):
                t = cp.tile([128, D // 128, no], bf16, tag=f"w_{name}")
                nc.sync.dma_start(out=t[:], in_=par[:])
                w_sb[name] = t
            bias_sb = {}
            for nm in ("b0row", "b1row", "b2row", "bw0row"):
                if nm in bias_p:
                    no = C if nm == "b2row" else D
                    t = cp.tile([128, no], f32, tag=f"bias_{nm}")
                    nc.sync.dma_start(out=t[:], in_=bias_p[nm][:])
                    bias_sb[nm] = t
            if "cn" in bias_p:
                t = cp.tile([128, cfg.BLK], f32)
                nc.sync.dma_start(out=t[:], in_=bias_p["cn"][:])
                bias_sb["cn"] = t

            self_qn = [0]
            layers = (
                ("wc", xt, h1s, h1f, True, "b0row"),
                ("w1", h1f, h2s, h2f, True, "b1row"),
                ("w2", h2f, None, None, False, "b2row"),
            )
            for li, (wname, srct, hshard, hfull, relu_ns, brow) in enumerate(layers):
                NO = D if li < 2 else C
                gdt = bf16 if li == 0 else f8
                wt = w_sb[wname]
                for ch in range(cfg.NCH):
                    nt = ch_tile0[ch + 1] - ch_tile0[ch]
                    ni = ch_idx0[ch + 1] - ch_idx0[ch]
                    code_ch = kp.tile([128, max_ch_tiles], bf16, tag="code_ch")
                    nc.sync.dma_start(
                        out=code_ch[:, :nt], in_=codes[:, ch_tile0[ch] : ch_tile0[ch + 1]]
                    )
                    idx_ch = kp.tile([128, max_ch_idx // 16], i16, tag="idx_ch")
                    nc.sync.dma_start(
                        out=idx_ch[:, : ni // 16],
                        in_=idxs[:, ch_idx0[ch] // 16 : ch_idx0[ch + 1] // 16],
                    )
                    for bl in range(cfg.CHB):
                        b = ch * cfg.CHB + bl
                        tb = int(T_b[b])
                        stag = sp.tile([128, max_T_b, D], gdt, tag="stag")
                        o = 0
                        for w in range(cfg.NW):
                            tw = int(T_bw[b, w])
                            if tw == 0:
                                continue
                            i0 = (int(grp_off[b * cfg.NW + w]) - ch_idx0[ch]) // 16
                            if isinstance(srct, list):
                                src_ap = srct[w][:]
                            else:
                                wlo = w * cfg.WIN
                                whi = min(wlo + cfg.WIN, cfg.RT)
                                src_ap = srct[wlo:whi, :]
                            # dma_gather caps at 1024 idxs (128 descs x 8 Q7
                            # cores); split and round-robin the 4 SWDGE queues
                            vb = int(Vmax[b * cfg.NW + w])
                            if vb < tw * 128:
                                # rows past the valid count are never written by
                                # the gather (trailing -1 idxs emit no
                                # descriptors); zero the partial tile so stale
                                # SBUF bits can't NaN-poison the PE
                                nc.vector.memset(stag[:, o + vb // 128, :], 0.0)
                            for t0 in range(0, tw, 8):
                                tn = min(8, tw - t0)
                                nvalid = min(tn * 128, vb - t0 * 128)
                                nc.gpsimd.dma_gather(
                                    out_ap=stag[:, o + t0 : o + t0 + tn, :],
                                    in_ap=src_ap,
                                    idxs_ap=idx_ch[:, i0 + t0 * 8 : i0 + (t0 + tn) * 8],
                                    num_idxs=tn * 128,
                                    num_idxs_reg=nvalid,
                                    elem_size=D,
                                    queue_num=self_qn[0] % 4,
                                )
                                self_qn[0] += 1
                            o += tw
                        acc = psA.tile([128, D], f32, space="PSUM", tag="acc")
                        cb = int(tile_off[b]) - ch_tile0[ch]
                        for g in range(0, tb, 4):
                            gn = min(4, tb - g)
                            oh = wp.tile([128, 4, 128], gdt, tag="oh")
                            nc.vector.tensor_tensor(
                                out=oh[:, :gn, :],
                                in0=code_ch[:, cb + g : cb + g + gn].to_broadcast(
                                    [128, gn, 128]
                                ),
                                in1=iota_sb[:, :gn, :],
                                op=mybir.AluOpType.is_equal,
                            )
                            for k in range(gn):
                                ti = g + k
                                nc.tensor.matmul(
                                    out=acc[:], lhsT=oh[:, k, :], rhs=stag[:, ti, :],
                                    start=(ti == 0), stop=(ti == tb - 1),
                                )
                        # scale by dst-degree norm, cast bf16
                        mn = wp.tile([128, D], bf16, tag="mn")
                        nc.vector.tensor_scalar_mul(mn[:], acc[:], nd_sb[:, b : b + 1])
                        # transpose mn -> [f_in, nodes] chunks
                        pT = psT.tile([128, D], bf16, space="PSUM", tag="pT")
                        for j in range(D // 128):
                            nc.tensor.transpose(
                                out=pT[:, j * 128 : (j + 1) * 128],
                                in_=mn[:, j * 128 : (j + 1) * 128],
                                identity=ident_sb[:],
                            )
                        lhsT = wp.tile([128, D], bf16, tag="lhsT")
                        nc.vector.tensor_copy(out=lhsT[:], in_=pT[:])
                        pd = psD.tile([128, NO], f32, space="PSUM", tag="pd")
                        for j in range(D // 128):
                            nc.tensor.matmul(
                                out=pd[:], lhsT=lhsT[:, j * 128 : (j + 1) * 128],
                                rhs=wt[:, j, :], start=(j == 0), stop=(j == D // 128 - 1),
                            )
                        # epilogue
                        pre = pd
                        if li == 0 and "bw0row" in bias_sb:
                            tmp = wp.tile([128, D], f32, tag="btmp")
                            nc.vector.tensor_scalar_mul(
                                tmp[:], bias_sb["bw0row"][:], bias_sb["cn"][:, b : b + 1]
                            )
                            nc.vector.tensor_add(pd[:], pd[:], tmp[:])
                        if brow in bias_sb:
                            nc.vector.tensor_add(pd[:], pd[:], bias_sb[brow][:])
                        if relu_ns:
                            ht = wp.tile([128, D], f8, tag="ht")
                            nc.scalar.activation(
                                ht[:], pre[:],
                                mybir.ActivationFunctionType.Relu,
                                scale=ns_sb[:, b : b + 1],
                            )
                            nc.sync.dma_start(
                                out=hshard[ch][bl * 128 : (bl + 1) * 128, :], in_=ht[:]
                            )
                        else:
                            ot = wp.tile([128, C], f32, tag="ot")
                            nc.vector.tensor_copy(out=ot[:], in_=pd[:])
                            nc.sync.dma_start(
                                out=out[b * 128 : (b + 1) * 128, :], in_=ot[:]
                            )
                    if li < 2:
                        nc.gpsimd.collective_compute(
                            "AllGather",
                            mybir.AluOpType.bypass,
                            ins=[hshard[ch][:]],
                            outs=[hfull[ch][:]],
                            replica_groups=[cores],
                        )
    nc.compile()
    return nc


_CACHE = {}


def _run(cfg, inputs, trace=False):
    pre = _preprocess(cfg, **inputs)
    bias_en = {
        "b0": bool(np.any(pre["bias"]["b0row"])),
        "b1": bool(np.any(pre["bias"]["b1row"])),
        "b2": bool(np.any(pre["bias"]["b2row"])),
        "blin": bool(np.any(pre["bias"]["bw0row"])),
    }
    key = (id(cfg), tuple(pre["T_bw"].reshape(-1)), tuple(pre["Vmax"]),
           tuple(sorted(bias_en.items())))
    if key not in _CACHE:
        _CACHE[key] = _build(cfg, pre["T_bw"], pre["Vmax"], bias_en)
    nc = _CACHE[key]

    in_maps = []
    for c in range(cfg.NC):
        m = dict(
            xt=pre["xt"],
            idxs=pre["idx"][c], codes=pre["codes"][c],
            wc=pre["wc"], w1=pre["w1"], w2=pre["w2"],
            nd=pre["nd"][c], ns=pre["ns"][c],
            iota=pre["iota"], ident=pre["ident"],
        )
        rep = lambda v: np.ascontiguousarray(np.broadcast_to(v[None, :], (128, v.shape[0])))
        if bias_en["b0"]:
            m["b0row"] = rep(pre["bias"]["b0row"])
        if bias_en["b1"]:
            m["b1row"] = rep(pre["bias"]["b1row"])
        if bias_en["b2"]:
            m["b2row"] = rep(pre["bias"]["b2row"])
        if bias_en["blin"]:
            m["bw0row"] = rep(pre["bias"]["bw0row"])
            m["cn"] = pre["bias"]["cn_t"][c]
        in_maps.append(m)

    r = run_bass_kernel_spmd(nc, in_maps, list(range(cfg.NC)), trace=trace)
    outs = [np.asarray(r.results[c]["out"])[: cfg.SH] for c in range(cfg.NC)]
    full = np.concatenate(outs, axis=0)[: cfg.N]
    return full, r


def kernel(**inputs):
    inputs = {k: np.asarray(v) for k, v in inputs.items()}
    out, _ = _run(CFG, inputs)
    return out
